# revision 34
# baseline (speedup 1.0000x reference)
"""Trainium2 Bass kernel for nn_Decoder_48052094107929 (moe_routing).

Data-parallel over 8 NeuronCores: batch B=8192 split into 8 shards of 1024
tokens; all weights replicated. Per core the whole decoder block runs as one
Tile kernel:

  phase 0: load x/y, layernorm(y) (token-major), PE-transpose x and ny to
           feature-major; gating softmax + top-2 -> c; broadcast c -> cb
  phase 2: qkv s = sum_e Wq'[e]@(c_e*ny) + Wkv'[e]@(c_e*x) + c@bq, fp8-e4m3
           DoubleRow matmuls over token halves (8 PSUM banks each);
           c-scaling muls run bf16 on DVE (2x/4x modes need all-2-byte
           operands) with the fp8 cast on the scalar engine (ny side) or
           directly on gpsimd (x side)
  phase 3: attention, batched across all 8 token tiles: gram via big DVE
           mul+reduce ops, no-max softmax (gram*SCALE <= ~13), combine via
           broadcast-AP muls; o transposed back to feature-major
  phase 4: fc1 fp8 DoubleRow, gelu fused into eviction
  phase 5: proj (bf16 - precision) + fc2 (fp8) share one PSUM accumulation
  phase 6: transpose + residual y + store

All big-matmul weights are pre-scaled x256 host-side (dodges fp8-e4m3
subnormals; exact in bf16) and every PSUM eviction applies scale 1/256.
LayerNorm gains/biases folded into weights host-side.
"""

import numpy as np
import ml_dtypes

import concourse.bass as bass
import concourse.mybir as mybir
import concourse.tile as tile
from concourse.bass_utils import run_bass_kernel_spmd
from concourse.masks import make_identity

# ---- problem constants (hardcoded per harness contract) ----
B = 8192
DIM = 1024
E = 4
H = 4
TOPK = 2
HD = DIM // H          # 256
SCALE = HD ** -0.5
HID = 4 * DIM          # 4096
EPS = 1e-5
NCORES = 8
B_C = B // NCORES      # 1024 tokens per core

F32 = mybir.dt.float32
BF16 = mybir.dt.bfloat16
F8 = mybir.dt.float8e4
AX = mybir.AxisListType
OP = mybir.AluOpType
AF = mybir.ActivationFunctionType
DR = mybir.MatmulPerfMode.DoubleRow

KD = DIM // 128        # 8  d-tiles
KD2 = KD // 2          # 4  d-pair-tiles
MH = HID // 128        # 32 hidden tiles
MH2 = MH // 2          # 16 hidden pair-tiles
NEG_BIG = -1.0e30
WS = 256.0             # weight pre-scale (fp8 subnormal dodge)
WSI = 1.0 / WS

FP8_FC1 = True
FP8_FC2 = True


def bf(a):
    return np.ascontiguousarray(a.astype(ml_dtypes.bfloat16))


def f32(a):
    return np.ascontiguousarray(a.astype(np.float32))


def f8(a):
    return np.ascontiguousarray(a.astype(ml_dtypes.float8_e4m3))


def pair8(wT, nf):
    """[d, f] weight (d=contraction) -> fp8 DoubleRow layout
    [KD2, 128, 2, nf], scaled by WS."""
    k2 = wT.shape[0] // 256
    return f8((wT * WS).reshape(k2, 2, 128, nf).transpose(0, 2, 1, 3))


def prep_weights(Wg, bg, Wqkv, Wp, bp, g1, bn1, g2, bn2, W1, bm1, W2, bm2):
    """Host-side, input-independent weight layout transforms."""
    Wq = Wqkv[:, :DIM, :]                        # [E, DIM, DIM] (f, d)
    Wk = Wqkv[:, DIM:2 * DIM, :]
    Wv = Wqkv[:, 2 * DIM:, :]
    Wqp = Wq * g1[None, None, :]                 # fold norm1 gamma into cols
    bq = np.einsum("efd,d->ef", Wq, bn1)         # [E, DIM] bias from norm1 beta
    Wkvs = Wk + Wv                               # aliasing bug: k+v share weights

    W1p = W1 * g2[None, :]
    bm1p = bm1 + W1 @ bn2

    out = {
        "wg": f32(Wg.T.reshape(KD, 128, E)),
        "bgv": f32(bg.reshape(1, E)),
        "bm1v": f32(bm1p.reshape(MH, 128).T),              # [128,32]
        "bpb2f": f32((bp + bm2).reshape(1, DIM)),          # [1,1024]
        "bq": bf((bq * WS).reshape(E, KD, 128)),
        # qkv fp8 DoubleRow weights
        "wq8": np.stack([pair8(Wqp[e].T, DIM) for e in range(E)]),
        "wkv8": np.stack([pair8(Wkvs[e].T, DIM) for e in range(E)]),
        # proj always bf16 (largest per-FLOP error contributor)
        "wp": bf(Wp.T.reshape(KD, 128, DIM) * WS),
    }
    if FP8_FC1:
        out["w18"] = np.ascontiguousarray(
            pair8(W1p.T, HID).transpose(1, 0, 2, 3))       # [128,4,2,4096]
    else:
        out["w1"] = bf(W1p.T.reshape(KD, 128, HID) * WS)
    if FP8_FC2:
        out["w28"] = pair8(W2.T, DIM)                      # [16,128,2,1024]
    else:
        out["w2"] = bf(W2.T.reshape(MH, 128, DIM) * WS)
    return out


def build_kernel(b_c=B_C):
    """Build the Bass module for one core processing b_c tokens."""
    nc = bass.Bass("TRN2", target_bir_lowering=False, debug=False)

    T = b_c // 128                 # token tiles
    TH = b_c // 2                  # tokens per half
    CHUNK = min(512, b_c)
    NCH = b_c // CHUNK
    T2 = T // 2

    # ---- DRAM tensors ----
    x_d = nc.dram_tensor("x", [b_c, DIM], F32, kind="ExternalInput")
    y_d = nc.dram_tensor("y", [b_c, DIM], F32, kind="ExternalInput")
    wq_d = nc.dram_tensor("wq8", [E, KD2, 128, 2, DIM], F8, kind="ExternalInput")
    wkv_d = nc.dram_tensor("wkv8", [E, KD2, 128, 2, DIM], F8,
                           kind="ExternalInput")
    wp_d = nc.dram_tensor("wp", [KD, 128, DIM], BF16, kind="ExternalInput")
    if FP8_FC1:
        w1_d = nc.dram_tensor("w18", [128, KD2, 2, HID], F8,
                              kind="ExternalInput")
    else:
        w1_d = nc.dram_tensor("w1", [KD, 128, HID], BF16, kind="ExternalInput")
    if FP8_FC2:
        w2_d = nc.dram_tensor("w28", [MH2, 128, 2, DIM], F8,
                              kind="ExternalInput")
    else:
        w2_d = nc.dram_tensor("w2", [MH, 128, DIM], BF16, kind="ExternalInput")
    wg_d = nc.dram_tensor("wg", [KD, 128, E], F32, kind="ExternalInput")
    bq_d = nc.dram_tensor("bq", [E, KD, 128], BF16, kind="ExternalInput")
    bg_d = nc.dram_tensor("bgv", [1, E], F32, kind="ExternalInput")
    bm1_d = nc.dram_tensor("bm1v", [128, MH], F32, kind="ExternalInput")
    bpb2_d = nc.dram_tensor("bpb2f", [1, DIM], F32, kind="ExternalInput")
    out_d = nc.dram_tensor("out", [b_c, DIM], F32, kind="ExternalOutput")
    csc_d = nc.dram_tensor("cscratch", [E, b_c], BF16, kind="Internal")

    x_r = x_d.ap().rearrange("(t p) d -> t p d", p=128)
    y_r = y_d.ap().rearrange("(t p) d -> t p d", p=128)
    out_r = out_d.ap().rearrange("(t p) d -> t p d", p=128)

    from contextlib import ExitStack

    with tile.TileContext(nc) as tc, ExitStack() as ctx0:
        consts = ctx0.enter_context(tc.tile_pool(name="consts", bufs=1))
        ident_bf = consts.tile([128, 128], BF16)
        make_identity(nc, ident_bf)
        ident_f = consts.tile([128, 128], F32)
        make_identity(nc, ident_f)
        eps_t = consts.tile([128, 1], F32)
        nc.vector.memset(eps_t, EPS)
        zero_t = consts.tile([128, 1], F32)
        nc.vector.memset(zero_t, 0.0)
        bg_sb = consts.tile([128, E], F32)
        nc.sync.dma_start(out=bg_sb, in_=bg_d.ap().to_broadcast([128, E]))
        wg_sb = consts.tile([128, KD, E], F32)
        nc.sync.dma_start(out=wg_sb, in_=wg_d.ap().rearrange("k p e -> p k e"))
        bq_sb = consts.tile([4, KD, 128], BF16)
        nc.sync.dma_start(out=bq_sb, in_=bq_d.ap())
        bm1_sb = consts.tile([128, MH], F32)
        nc.sync.dma_start(out=bm1_sb, in_=bm1_d.ap())
        bpb2_sb = consts.tile([128, DIM], F32)
        nc.sync.dma_start(out=bpb2_sb,
                          in_=bpb2_d.ap().to_broadcast([128, DIM]))

        ny8_p = ctx0.enter_context(tc.tile_pool(name="ny8", bufs=1))
        ny8 = ny8_p.tile([128, KD, b_c], F8 if FP8_FC1 else BF16)
        oT_p = ctx0.enter_context(tc.tile_pool(name="oT", bufs=1))
        oT = oT_p.tile([128, KD, b_c], BF16)
        sT_p = ctx0.enter_context(tc.tile_pool(name="sT", bufs=1))
        sT = sT_p.tile([128, KD, b_c], BF16)

        with ExitStack() as ctxa:
            nyT_p = ctxa.enter_context(tc.tile_pool(name="nyT", bufs=1))
            nyT = nyT_p.tile([128, KD, b_c], BF16)
            xT_p = ctxa.enter_context(tc.tile_pool(name="xT", bufs=1))
            xT = xT_p.tile([128, KD, b_c], BF16)
            cb_p = ctxa.enter_context(tc.tile_pool(name="cb", bufs=1))
            cb = cb_p.tile([128, E, b_c], BF16)
            crows_p = ctxa.enter_context(tc.tile_pool(name="crows", bufs=1))
            crows = crows_p.tile([4, b_c], BF16)

            # ---------- phase 0: load x/y, layernorm(y), transposes ----------
            # Per token-half: x transposes + gating first (qkv needs cb),
            # then ny. Gating softmax/top-2 is batched across the half's 4
            # tiles in wide DVE ops (small-op fixed costs dominate
            # otherwise); gating logits are O(3) so exp needs no max-sub.
            with ExitStack() as ctx_p0:
                xin = ctx_p0.enter_context(tc.tile_pool(name="xin", bufs=3))
                yin = ctx_p0.enter_context(tc.tile_pool(name="yin", bufs=3))
                nrm = ctx_p0.enter_context(tc.tile_pool(name="nrm", bufs=3))
                stat = ctx_p0.enter_context(tc.tile_pool(name="stat", bufs=2))
                gsmall = ctx_p0.enter_context(tc.tile_pool(name="gsm", bufs=2))
                xf = ctx_p0.enter_context(tc.tile_pool(name="xf", bufs=2))
                tp_ps = ctx_p0.enter_context(
                    tc.tile_pool(name="tp_ps", bufs=2, space="PSUM"))
                tpb_ps = ctx_p0.enter_context(
                    tc.tile_pool(name="tpb_ps", bufs=2, space="PSUM"))
                g_ps = ctx_p0.enter_context(
                    tc.tile_pool(name="g_ps", bufs=2, space="PSUM"))
                cr_ps = ctx_p0.enter_context(
                    tc.tile_pool(name="cr_ps", bufs=1, space="PSUM"))
                crows_ps = cr_ps.tile([4, b_c], F32)

                for hb in range(2):
                    tiles = range(hb * T2, (hb + 1) * T2)
                    # --- x loads, f32 transposes, gating matmuls ---
                    gps_all = g_ps.tile([128, T2, E], F32, tag="gpsa")
                    xts = []
                    for i, t in enumerate(tiles):
                        xt = xin.tile([128, DIM], F32, tag=f"xt{i}")
                        nc.sync.dma_start(out=xt, in_=x_r[t])
                        xts.append(xt)
                        xf_t = xf.tile([128, KD, 128], F32, tag="xf_t")
                        for grp in range(KD // 4):
                            pst = tp_ps.tile([128, 4, 128], F32, tag="tp")
                            for j in range(4):
                                kd = grp * 4 + j
                                nc.tensor.transpose(
                                    pst[:, j, :],
                                    xt[:, kd * 128:(kd + 1) * 128], ident_f)
                            gsl = slice(grp * 4, (grp + 1) * 4)
                            nc.scalar.copy(
                                out=xT[:, gsl, t * 128:(t + 1) * 128], in_=pst)
                            nc.scalar.copy(out=xf_t[:, gsl, :], in_=pst)
                        for kd in range(KD):
                            nc.tensor.matmul(gps_all[:, i, :], xf_t[:, kd, :],
                                             wg_sb[:, kd, :],
                                             start=(kd == 0),
                                             stop=(kd == KD - 1))
                    # --- batched gating softmax + top-2 over the half ---
                    bgb = (bg_sb.rearrange("p (o e) -> p o e", o=1)
                           .to_broadcast([128, T2, E]))
                    glog = gsmall.tile([128, T2, E], F32, tag="glog")
                    nc.vector.tensor_add(glog, gps_all, bgb)
                    gexp = gsmall.tile([128, T2, E], F32, tag="gexp")
                    nc.scalar.activation(out=gexp, in_=glog, func=AF.Exp,
                                         bias=zero_t, scale=1.0)
                    gden = gsmall.tile([128, T2, 1], F32, tag="gden")
                    nc.vector.reduce_sum(gden, gexp, AX.X)
                    grec = gsmall.tile([128, T2, 1], F32, tag="grec")
                    nc.vector.reciprocal(out=grec, in_=gden)
                    gate = gsmall.tile([128, T2, E], F32, tag="gate")
                    nc.vector.tensor_mul(gate, gexp,
                                         grec.to_broadcast([128, T2, E]))
                    m1 = gsmall.tile([128, T2, 1], F32, tag="m1")
                    nc.vector.tensor_reduce(out=m1, in_=gate, axis=AX.X,
                                            op=OP.max)
                    eq1 = gsmall.tile([128, T2, E], F32, tag="eq1")
                    nc.vector.tensor_tensor(out=eq1, in0=gate,
                                            in1=m1.to_broadcast([128, T2, E]),
                                            op=OP.is_equal)
                    msk = gsmall.tile([128, T2, E], F32, tag="msk")
                    nc.vector.scalar_tensor_tensor(out=msk, in0=eq1,
                                                   scalar=NEG_BIG, in1=gate,
                                                   op0=OP.mult, op1=OP.add)
                    m2 = gsmall.tile([128, T2, 1], F32, tag="m2")
                    nc.vector.tensor_reduce(out=m2, in_=msk, axis=AX.X,
                                            op=OP.max)
                    keep = gsmall.tile([128, T2, E], F32, tag="keep")
                    nc.vector.tensor_tensor(out=keep, in0=gate,
                                            in1=m2.to_broadcast([128, T2, E]),
                                            op=OP.is_ge)
                    c_all = gsmall.tile([128, T2, E], F32, tag="c_all")
                    nc.vector.tensor_mul(c_all, gate, keep)
                    for i, t in enumerate(tiles):
                        nc.tensor.transpose(
                            crows_ps[:, t * 128:(t + 1) * 128],
                            c_all[:, i, :], ident_f)
                    hsl = slice(hb * TH, (hb + 1) * TH)
                    nc.vector.tensor_copy(out=crows[:, hsl],
                                          in_=crows_ps[:, hsl])
                    nc.sync.dma_start(out=csc_d.ap()[:, hsl],
                                      in_=crows[:, hsl])
                    for e in range(E):
                        nc.sync.dma_start(
                            out=cb[:, e, hsl],
                            in_=csc_d.ap()[e:e + 1, hsl]
                            .to_broadcast([128, TH]))

                # second pass: layernorm(y) + bf16 transposes per half —
                # emitted after BOTH halves' gating so the half-1 gating
                # vector chain isn't queued behind half-0's LN on DVE
                for hb in range(2):
                    tiles = range(hb * T2, (hb + 1) * T2)
                    mv_all = stat.tile([128, T2, 2], F32, tag="mv_all")
                    yts0 = []
                    for i, t in enumerate(tiles):
                        yt = yin.tile([128, DIM], F32, tag=f"yt{i}")
                        nc.sync.dma_start(out=yt, in_=y_r[t])
                        yts0.append(yt)
                        st6 = stat.tile([128, 2, 6], F32, tag="st6")
                        yv = yt.rearrange("p (s d) -> p s d", s=2)
                        for s in range(2):
                            nc.vector.bn_stats(out=st6[:, s, :], in_=yv[:, s, :])
                        nc.vector.bn_aggr(out=mv_all[:, i, :], in_=st6)
                    sd = stat.tile([128, T2], F32, tag="sd")
                    nc.scalar.activation(out=sd, in_=mv_all[:, :, 1],
                                         func=AF.Sqrt, bias=eps_t, scale=1.0)
                    rstd = stat.tile([128, T2], F32, tag="rstd")
                    nc.vector.reciprocal(out=rstd, in_=sd)
                    for i, t in enumerate(tiles):
                        ny = nrm.tile([128, DIM], BF16, tag="ny")
                        nc.vector.tensor_scalar(out=ny, in0=yts0[i],
                                                scalar1=mv_all[:, i, 0:1],
                                                scalar2=rstd[:, i:i + 1],
                                                op0=OP.subtract, op1=OP.mult)
                        for grp in range(KD // 4):
                            pstb = tpb_ps.tile([128, 4, 128], BF16, tag="tpb")
                            for j in range(4):
                                kd = grp * 4 + j
                                nc.tensor.transpose(
                                    pstb[:, j, :],
                                    ny[:, kd * 128:(kd + 1) * 128], ident_bf)
                            gsl = slice(grp * 4, (grp + 1) * 4)
                            tsl128 = slice(t * 128, (t + 1) * 128)
                            nc.vector.tensor_copy(out=nyT[:, gsl, tsl128],
                                                  in_=pstb)
                            nc.scalar.copy(out=ny8[:, gsl, tsl128], in_=pstb)

            # ---------- phase 2: qkv fp8 DoubleRow over token halves ----------
            with ExitStack() as ctx_p2:
                wstr = ctx_p2.enter_context(tc.tile_pool(name="wstr", bufs=6))
                scl8 = ctx_p2.enter_context(tc.tile_pool(name="scl8", bufs=6))
                qk_ps = ctx_p2.enter_context(
                    tc.tile_pool(name="qk_ps", bufs=1, space="PSUM"))
                for th in range(2):
                    tsl = slice(th * TH, (th + 1) * TH)
                    ps = [qk_ps.tile([128, TH], F32, tag=f"qk{m}",
                                     name=f"qk{m}_{th}") for m in range(KD)]
                    step = 0
                    for e in range(E):
                        for k2 in range(KD2):
                            for which, w_d2 in enumerate((wq_d, wkv_d)):
                                wt8 = wstr.tile([128, 2, DIM], F8, tag="wt8")
                                nc.sync.dma_start(out=wt8, in_=w_d2.ap()[e, k2])
                                act = xT if which else nyT
                                seng = nc.gpsimd if which else nc.vector
                                cbb = (cb[:, e:e + 1, tsl]
                                       .to_broadcast([128, 2, TH]))
                                sc8 = scl8.tile([128, 2, TH], F8,
                                                tag=f"sc8{which}")
                                seng.tensor_mul(
                                    sc8, act[:, 2 * k2:2 * k2 + 2, tsl], cbb)
                                for m in range(KD):
                                    nc.tensor.matmul(
                                        ps[m],
                                        wt8[:, :, m * 128:(m + 1) * 128],
                                        sc8,
                                        start=(step == 0), stop=False,
                                        perf_mode=DR)
                                step += 1
                    # bias step: sum_e c[e,t] * bq[e,f] (bf16, normal mode)
                    for m in range(KD):
                        nc.tensor.matmul(ps[m], bq_sb[:, m, :], crows[:, tsl],
                                         start=False, stop=True)
                    for m in range(KD):
                        nc.scalar.activation(out=sT[:, m, tsl], in_=ps[m],
                                             func=AF.Identity, bias=zero_t,
                                             scale=WSI)

        # ---- phases 3-6. PE executes in program order, so emission order
        # IS the PE schedule: s-transposes, fc1, fc2 (needs only hT), THEN
        # the combine-dependent o-transposes, proj, final transposes. The
        # attention combine (vector) hides under fc1+fc2's PE time. ----
        with ExitStack() as ctxb:
            hT_p = ctxb.enter_context(tc.tile_pool(name="hT", bufs=1))
            hT = hT_p.tile([128, MH, b_c], F8 if FP8_FC2 else BF16)
            # at_ps/f1_ps stay open through phase 5 so f2_ps's 4 banks can
            # only alias qkv banks (readers long done) — NOT the transpose
            # banks whose last reader waits on the attention combine (that
            # WAR chain would serialize fc2 behind the combine)
            at_ps = ctxb.enter_context(
                tc.tile_pool(name="at_ps", bufs=2, space="PSUM"))
            f1_ps = ctxb.enter_context(
                tc.tile_pool(name="f1_ps", bufs=2, space="PSUM"))
            stok_p = ctxb.enter_context(tc.tile_pool(name="stok", bufs=1))
            asm = ctxb.enter_context(tc.tile_pool(name="asm", bufs=1))
            scr = ctxb.enter_context(tc.tile_pool(name="scr", bufs=1))
            oac_p = ctxb.enter_context(tc.tile_pool(name="oac", bufs=2))
            with ExitStack() as ctx_p3:
                w1str = ctx_p3.enter_context(tc.tile_pool(name="w1str", bufs=4))

                # s^T -> token-major s_all (all tiles)
                s_all = stok_p.tile([128, T, DIM], BF16)
                for t in range(T):
                    for grp in range(KD // 4):
                        pst = at_ps.tile([128, 4, 128], BF16, tag="atp")
                        for j in range(4):
                            mf = grp * 4 + j
                            nc.tensor.transpose(
                                pst[:, j, :],
                                sT[:, mf, t * 128:(t + 1) * 128], ident_bf)
                        nc.vector.tensor_copy(
                            out=s_all[:, t, grp * 512:(grp + 1) * 512],
                            in_=pst)

                # batched gram: for each head pair, big mul + reduce
                gram = asm.tile([128, T, H * H], F32)
                for h in range(H):
                    for g in range(h, H):
                        prod = scr.tile([128, T, HD], BF16, tag="prod")
                        nc.vector.tensor_mul(
                            prod, s_all[:, :, h * HD:(h + 1) * HD],
                            s_all[:, :, g * HD:(g + 1) * HD])
                        nc.vector.reduce_sum(
                            gram[:, :, h * H + g:h * H + g + 1], prod, AX.X)
                        if g != h:
                            nc.vector.tensor_copy(
                                out=gram[:, :, g * H + h:g * H + h + 1],
                                in_=gram[:, :, h * H + g:h * H + g + 1])
                # no-max softmax: gram*SCALE bounded (~13), exp is f32-safe
                pexp = asm.tile([128, T, H * H], F32)
                nc.scalar.activation(out=pexp, in_=gram, func=AF.Exp,
                                     bias=zero_t, scale=SCALE)
                den = asm.tile([128, T, H], F32)
                nc.vector.reduce_sum(
                    den, pexp.rearrange("p t (h g) -> p t h g", h=H), AX.X)
                rden = asm.tile([128, T, H], F32)
                nc.vector.reciprocal(out=rden, in_=den)
                an = asm.tile([128, T, H, H], BF16)
                nc.vector.tensor_mul(
                    an, pexp.rearrange("p t (h g) -> p t h g", h=H),
                    rden.rearrange("p t (h o) -> p t h o", o=1)
                    .to_broadcast([128, T, H, H]))

                # ---------- phase 4: MLP fc1 (gelu fused into eviction) ----
                if FP8_FC1:
                    for mh in range(MH):
                        w1t8 = w1str.tile([128, KD2, 2, 128], F8, tag="w1t8")
                        nc.sync.dma_start(
                            out=w1t8,
                            in_=w1_d.ap()[:, :, :, mh * 128:(mh + 1) * 128])
                        for ch in range(NCH):
                            psf = f1_ps.tile([128, CHUNK], F32, tag="psf")
                            for k2 in range(KD2):
                                nc.tensor.matmul(
                                    psf, w1t8[:, k2, :, :],
                                    ny8[:, 2 * k2:2 * k2 + 2,
                                        ch * CHUNK:(ch + 1) * CHUNK],
                                    start=(k2 == 0), stop=(k2 == KD2 - 1),
                                    perf_mode=DR)
                            nc.scalar.activation(
                                out=hT[:, mh, ch * CHUNK:(ch + 1) * CHUNK],
                                in_=psf, func=AF.Gelu,
                                bias=bm1_sb[:, mh:mh + 1], scale=WSI)
                else:
                    for mh in range(MH):
                        w1t = w1str.tile([128, KD, 128], BF16, tag="w1t")
                        nc.sync.dma_start(
                            out=w1t,
                            in_=w1_d.ap()[:, :, mh * 128:(mh + 1) * 128]
                            .rearrange("k p f -> p k f"))
                        for ch in range(NCH):
                            psf = f1_ps.tile([128, CHUNK], F32, tag="psf")
                            for kd in range(KD):
                                nc.tensor.matmul(
                                    psf, w1t[:, kd, :],
                                    ny8[:, kd, ch * CHUNK:(ch + 1) * CHUNK],
                                    start=(kd == 0), stop=(kd == KD - 1))
                            nc.scalar.activation(
                                out=hT[:, mh, ch * CHUNK:(ch + 1) * CHUNK],
                                in_=psf, func=AF.Gelu,
                                bias=bm1_sb[:, mh:mh + 1], scale=WSI)

            # ---------- phase 5: fc2, then combine+o-transposes, then proj —
            # all token-major (activations stationary, weights moving), so
            # the result needs no final transpose and streams straight out.
            # Token-quarters of 2 tiles keep PSUM at 4 banks (+at/f1 = 8).
            with ExitStack() as ctx_p5:
                w2str = ctx_p5.enter_context(tc.tile_pool(name="w2str", bufs=4))
                res2_p = ctx_p5.enter_context(tc.tile_pool(name="res2", bufs=1))
                outok_p = ctx_p5.enter_context(tc.tile_pool(name="outok", bufs=2))
                ytail = ctx_p5.enter_context(tc.tile_pool(name="ytail", bufs=1))
                f2_ps = ctx_p5.enter_context(
                    tc.tile_pool(name="f2_ps", bufs=1, space="PSUM"))
                res2 = res2_p.tile([128, T, DIM], BF16)

                # prefetch residual-y tiles
                yts = []
                for t in range(T):
                    yt2 = ytail.tile([128, DIM], F32, tag=f"yt{t}",
                                     name=f"yt2_{t}")
                    nc.sync.dma_start(out=yt2, in_=y_r[t])
                    yts.append(yt2)

                # --- stage A: fc2 token-major, evict partial + bias ---
                for q in range(4):          # quarters of 2 token tiles
                    pq = [f2_ps.tile([128, DIM], F32, tag=f"f2_{i}",
                                     name=f"f2_{i}_{q}") for i in range(2)]
                    for kh2 in range(MH2):
                        w2t8 = w2str.tile([128, 2, DIM], F8, tag="w2t8")
                        nc.sync.dma_start(out=w2t8, in_=w2_d.ap()[kh2])
                        for i in range(2):
                            t = 2 * q + i
                            lh = hT[:, 2 * kh2:2 * kh2 + 2,
                                    t * 128:(t + 1) * 128]
                            for fh in range(2):
                                nc.tensor.matmul(
                                    pq[i][:, fh * 512:(fh + 1) * 512],
                                    lh, w2t8[:, :, fh * 512:(fh + 1) * 512],
                                    start=(kh2 == 0),
                                    stop=(kh2 == MH2 - 1),
                                    perf_mode=DR)
                    for i in range(2):
                        t = 2 * q + i
                        nc.vector.scalar_tensor_tensor(
                            out=res2[:, t, :], in0=pq[i], scalar=WSI,
                            in1=bpb2_sb, op0=OP.mult, op1=OP.add)

                # --- stage B: attention combine + o -> oT transposes ---
                # (emitted after fc2 so these combine-dependent PE ops queue
                # behind fc2's matmuls, not in front of them)
                for th2 in range(2):
                    t2sl = slice(th2 * T2, (th2 + 1) * T2)
                    o_acc = oac_p.tile([128, T2, DIM], BF16, tag="oacc")
                    ov = o_acc.rearrange("p t (h d) -> p t h d", h=H)
                    for g in range(H):
                        sgb = (s_all[:, t2sl, g * HD:(g + 1) * HD]
                               .rearrange("p t (o d) -> p t o d", o=1)
                               .to_broadcast([128, T2, H, HD]))
                        agb = (an[:, t2sl, :, g:g + 1]
                               .to_broadcast([128, T2, H, HD]))
                        if g == 0:
                            nc.vector.tensor_mul(ov, sgb, agb)
                        else:
                            tmp = scr.tile([128, T2, H, HD], BF16, tag="ctmp")
                            nc.vector.tensor_mul(tmp, sgb, agb)
                            nc.vector.tensor_add(ov, ov, tmp)
                    for i in range(T2):
                        t = th2 * T2 + i
                        for grp in range(KD // 4):
                            pst = at_ps.tile([128, 4, 128], BF16, tag="atp")
                            for j in range(4):
                                kd = grp * 4 + j
                                nc.tensor.transpose(
                                    pst[:, j, :],
                                    o_acc[:, i, kd * 128:(kd + 1) * 128],
                                    ident_bf)
                            nc.vector.tensor_copy(
                                out=oT[:, grp * 4:(grp + 1) * 4,
                                       t * 128:(t + 1) * 128],
                                in_=pst)

                # --- stage C: proj token-major; merge + residual + store ---
                for q in range(4):
                    pp = [f2_ps.tile([128, DIM], F32, tag=f"f2_{i}",
                                     name=f"pj_{i}_{q}") for i in range(2)]
                    for kd in range(KD):
                        wpt = w2str.tile([128, DIM], BF16, tag="wpt")
                        nc.sync.dma_start(out=wpt, in_=wp_d.ap()[kd])
                        for i in range(2):
                            t = 2 * q + i
                            lo = oT[:, kd, t * 128:(t + 1) * 128]
                            for fh in range(2):
                                nc.tensor.matmul(
                                    pp[i][:, fh * 512:(fh + 1) * 512],
                                    lo, wpt[:, fh * 512:(fh + 1) * 512],
                                    start=(kd == 0), stop=(kd == KD - 1))
                    for i in range(2):
                        t = 2 * q + i
                        out_tok = outok_p.tile([128, DIM], F32, tag="out_tok")
                        nc.vector.scalar_tensor_tensor(
                            out=out_tok, in0=pp[i], scalar=WSI,
                            in1=res2[:, t, :], op0=OP.mult, op1=OP.add)
                        nc.vector.tensor_tensor(out=out_tok, in0=out_tok,
                                                in1=yts[t], op=OP.add)
                        nc.sync.dma_start(out=out_r[t], in_=out_tok)

    return nc


MAX_WAITS = 1


def split_big_waits(nc, limit=MAX_WAITS):
    """Walrus rejects instructions carrying too many sem waits; move the
    overflow onto preceding single-wait NoOps on the same engine."""
    n = 0
    for fn in nc.m.functions:
        for blk in fn.blocks:
            new_insts = []
            for inst in blk.instructions:
                si = inst.sync_info
                if si is not None and len(si.on_wait) > limit:
                    waits = list(si.on_wait)
                    while len(waits) > limit:
                        w, waits = waits[0], waits[1:]
                        nop = mybir.InstNoOp(name=f"WSPLIT-{nc.next_id()}")
                        nop.engine = inst.engine
                        nop.sync_info = mybir.SyncInfo(on_wait=[w], on_update=[])
                        new_insts.append(nop)
                        n += 1
                    si.on_wait = waits
                new_insts.append(inst)
            blk.instructions[:] = new_insts
    return n


_NC_CACHE = {}


def get_nc(b_c=B_C):
    """Build + apply the walrus wait-split workaround (HW compile path)."""
    if b_c not in _NC_CACHE:
        nc = build_kernel(b_c)
        split_big_waits(nc)
        _NC_CACHE[b_c] = nc
    return _NC_CACHE[b_c]


def make_in_maps(inputs, b_c=B_C, ncores=NCORES):
    w = prep_weights(
        np.asarray(inputs["Wg"]), np.asarray(inputs["bg"]),
        np.asarray(inputs["Wqkv"]), np.asarray(inputs["Wp"]),
        np.asarray(inputs["bp"]), np.asarray(inputs["g1"]),
        np.asarray(inputs["bn1"]), np.asarray(inputs["g2"]),
        np.asarray(inputs["bn2"]), np.asarray(inputs["W1"]),
        np.asarray(inputs["bm1"]), np.asarray(inputs["W2"]),
        np.asarray(inputs["bm2"]))
    x = f32(np.asarray(inputs["x"]))
    y = f32(np.asarray(inputs["y"]))
    in_maps = []
    for c in range(ncores):
        sl = slice(c * b_c, (c + 1) * b_c)
        in_maps.append({"x": x[sl], "y": y[sl], **w})
    return in_maps


def kernel(**inputs):
    nc = get_nc(B_C)
    in_maps = make_in_maps(inputs)
    res = run_bass_kernel_spmd(nc, in_maps, core_ids=list(range(NCORES)))
    return np.concatenate([res.results[c]["out"] for c in range(NCORES)], axis=0)


# revision 46
# speedup vs baseline: 1.0640x; 1.0640x over previous
"""Trainium2 Bass kernel for nn_Decoder_48052094107929 (moe_routing).

Data-parallel over 8 NeuronCores: batch B=8192 split into 8 shards of 1024
tokens; all weights replicated. Per core the whole decoder block runs as one
Tile kernel:

  phase 0: load x/y, layernorm(y) (token-major), PE-transpose x and ny to
           feature-major; gating softmax + top-2 -> c; broadcast c -> cb
  phase 2: qkv s = sum_e Wq'[e]@(c_e*ny) + Wkv'[e]@(c_e*x) + c@bq, fp8-e4m3
           DoubleRow matmuls over token halves (8 PSUM banks each);
           c-scaling muls run bf16 on DVE (2x/4x modes need all-2-byte
           operands) with the fp8 cast on the scalar engine (ny side) or
           directly on gpsimd (x side)
  phase 3: attention, batched across all 8 token tiles: gram via big DVE
           mul+reduce ops, no-max softmax (gram*SCALE <= ~13), combine via
           broadcast-AP muls; o transposed back to feature-major
  phase 4: fc1 fp8 DoubleRow, gelu fused into eviction
  phase 5: proj (bf16 - precision) + fc2 (fp8) share one PSUM accumulation
  phase 6: transpose + residual y + store

All big-matmul weights are pre-scaled x256 host-side (dodges fp8-e4m3
subnormals; exact in bf16) and every PSUM eviction applies scale 1/256.
LayerNorm gains/biases folded into weights host-side.
"""

import numpy as np
import ml_dtypes

import concourse.bass as bass
import concourse.mybir as mybir
import concourse.tile as tile
from concourse.bass_utils import run_bass_kernel_spmd
from concourse.masks import make_identity

# ---- problem constants (hardcoded per harness contract) ----
B = 8192
DIM = 1024
E = 4
H = 4
TOPK = 2
HD = DIM // H          # 256
SCALE = HD ** -0.5
HID = 4 * DIM          # 4096
EPS = 1e-5
NCORES = 8
B_C = B // NCORES      # 1024 tokens per core

F32 = mybir.dt.float32
BF16 = mybir.dt.bfloat16
F8 = mybir.dt.float8e4
AX = mybir.AxisListType
OP = mybir.AluOpType
AF = mybir.ActivationFunctionType
DR = mybir.MatmulPerfMode.DoubleRow

KD = DIM // 128        # 8  d-tiles
KD2 = KD // 2          # 4  d-pair-tiles
MH = HID // 128        # 32 hidden tiles
MH2 = MH // 2          # 16 hidden pair-tiles
NEG_BIG = -1.0e30
WS = 256.0             # weight pre-scale (fp8 subnormal dodge)
WSI = 1.0 / WS

FP8_FC1 = True
FP8_FC2 = True


def bf(a):
    return np.ascontiguousarray(a.astype(ml_dtypes.bfloat16))


def f32(a):
    return np.ascontiguousarray(a.astype(np.float32))


def f8(a):
    return np.ascontiguousarray(a.astype(ml_dtypes.float8_e4m3))


def pair8(wT, nf):
    """[d, f] weight (d=contraction) -> fp8 DoubleRow layout
    [KD2, 128, 2, nf], scaled by WS."""
    k2 = wT.shape[0] // 256
    return f8((wT * WS).reshape(k2, 2, 128, nf).transpose(0, 2, 1, 3))


def prep_weights(Wg, bg, Wqkv, Wp, bp, g1, bn1, g2, bn2, W1, bm1, W2, bm2):
    """Host-side, input-independent weight layout transforms."""
    Wq = Wqkv[:, :DIM, :]                        # [E, DIM, DIM] (f, d)
    Wk = Wqkv[:, DIM:2 * DIM, :]
    Wv = Wqkv[:, 2 * DIM:, :]
    Wqp = Wq * g1[None, None, :]                 # fold norm1 gamma into cols
    bq = np.einsum("efd,d->ef", Wq, bn1)         # [E, DIM] bias from norm1 beta
    Wkvs = Wk + Wv                               # aliasing bug: k+v share weights

    W1p = W1 * g2[None, :]
    bm1p = bm1 + W1 @ bn2

    out = {
        "wg": f32(Wg.T.reshape(KD, 128, E)),
        "bgv": f32(bg.reshape(1, E)),
        "bm1v": f32(bm1p.reshape(MH, 128).T),              # [128,32]
        "bpb2": f32((bp + bm2).reshape(KD, 128).T),        # [128,8]
        "bq": bf((bq * WS).reshape(E, KD, 128)),
        # qkv fp8 DoubleRow weights
        "wq8": np.stack([pair8(Wqp[e].T, DIM) for e in range(E)]),
        "wkv8": np.stack([pair8(Wkvs[e].T, DIM) for e in range(E)]),
        # proj always bf16 (largest per-FLOP error contributor)
        "wp": bf(Wp.T.reshape(KD, 128, DIM) * WS),
    }
    if FP8_FC1:
        out["w18"] = np.ascontiguousarray(
            pair8(W1p.T, HID).transpose(1, 0, 2, 3))       # [128,4,2,4096]
    else:
        out["w1"] = bf(W1p.T.reshape(KD, 128, HID) * WS)
    if FP8_FC2:
        out["w28"] = pair8(W2.T, DIM)                      # [16,128,2,1024]
    else:
        out["w2"] = bf(W2.T.reshape(MH, 128, DIM) * WS)
    return out


def build_kernel(b_c=B_C):
    """Build the Bass module for one core processing b_c tokens."""
    nc = bass.Bass("TRN2", target_bir_lowering=False, debug=False)

    T = b_c // 128                 # token tiles
    TH = b_c // 2                  # tokens per half
    CHUNK = min(512, b_c)
    NCH = b_c // CHUNK
    T2 = T // 2

    # ---- DRAM tensors ----
    x_d = nc.dram_tensor("x", [b_c, DIM], F32, kind="ExternalInput")
    y_d = nc.dram_tensor("y", [b_c, DIM], F32, kind="ExternalInput")
    wq_d = nc.dram_tensor("wq8", [E, KD2, 128, 2, DIM], F8, kind="ExternalInput")
    wkv_d = nc.dram_tensor("wkv8", [E, KD2, 128, 2, DIM], F8,
                           kind="ExternalInput")
    wp_d = nc.dram_tensor("wp", [KD, 128, DIM], BF16, kind="ExternalInput")
    if FP8_FC1:
        w1_d = nc.dram_tensor("w18", [128, KD2, 2, HID], F8,
                              kind="ExternalInput")
    else:
        w1_d = nc.dram_tensor("w1", [KD, 128, HID], BF16, kind="ExternalInput")
    if FP8_FC2:
        w2_d = nc.dram_tensor("w28", [MH2, 128, 2, DIM], F8,
                              kind="ExternalInput")
    else:
        w2_d = nc.dram_tensor("w2", [MH, 128, DIM], BF16, kind="ExternalInput")
    wg_d = nc.dram_tensor("wg", [KD, 128, E], F32, kind="ExternalInput")
    bq_d = nc.dram_tensor("bq", [E, KD, 128], BF16, kind="ExternalInput")
    bg_d = nc.dram_tensor("bgv", [1, E], F32, kind="ExternalInput")
    bm1_d = nc.dram_tensor("bm1v", [128, MH], F32, kind="ExternalInput")
    bpb2_d = nc.dram_tensor("bpb2", [128, KD], F32, kind="ExternalInput")
    out_d = nc.dram_tensor("out", [b_c, DIM], F32, kind="ExternalOutput")
    csc_d = nc.dram_tensor("cscratch", [E, b_c], BF16, kind="Internal")

    x_r = x_d.ap().rearrange("(t p) d -> t p d", p=128)
    y_r = y_d.ap().rearrange("(t p) d -> t p d", p=128)
    out_r = out_d.ap().rearrange("(t p) d -> t p d", p=128)

    from contextlib import ExitStack

    with tile.TileContext(nc) as tc, ExitStack() as ctx0:
        consts = ctx0.enter_context(tc.tile_pool(name="consts", bufs=1))
        ident_bf = consts.tile([128, 128], BF16)
        make_identity(nc, ident_bf)
        ident_f = consts.tile([128, 128], F32)
        make_identity(nc, ident_f)
        eps_t = consts.tile([128, 1], F32)
        nc.vector.memset(eps_t, EPS)
        zero_t = consts.tile([128, 1], F32)
        nc.vector.memset(zero_t, 0.0)
        bg_sb = consts.tile([128, E], F32)
        nc.sync.dma_start(out=bg_sb, in_=bg_d.ap().to_broadcast([128, E]))
        wg_sb = consts.tile([128, KD, E], F32)
        nc.sync.dma_start(out=wg_sb, in_=wg_d.ap().rearrange("k p e -> p k e"))
        bq_sb = consts.tile([4, KD, 128], BF16)
        nc.sync.dma_start(out=bq_sb, in_=bq_d.ap())
        bm1_sb = consts.tile([128, MH], F32)
        nc.sync.dma_start(out=bm1_sb, in_=bm1_d.ap())
        bpb2_sb = consts.tile([128, KD], F32)
        nc.sync.dma_start(out=bpb2_sb, in_=bpb2_d.ap())

        ny8_p = ctx0.enter_context(tc.tile_pool(name="ny8", bufs=1))
        ny8 = ny8_p.tile([128, KD, b_c], F8 if FP8_FC1 else BF16)
        oT_p = ctx0.enter_context(tc.tile_pool(name="oT", bufs=1))
        oT = oT_p.tile([128, KD, b_c], BF16)
        sT_p = ctx0.enter_context(tc.tile_pool(name="sT", bufs=1))
        sT = sT_p.tile([128, KD, b_c], BF16)

        with ExitStack() as ctxa:
            nyT_p = ctxa.enter_context(tc.tile_pool(name="nyT", bufs=1))
            nyT = nyT_p.tile([128, KD, b_c], BF16)
            xT_p = ctxa.enter_context(tc.tile_pool(name="xT", bufs=1))
            xT = xT_p.tile([128, KD, b_c], BF16)
            cb_p = ctxa.enter_context(tc.tile_pool(name="cb", bufs=1))
            cb = cb_p.tile([128, E, b_c], BF16)
            crows_p = ctxa.enter_context(tc.tile_pool(name="crows", bufs=1))
            crows = crows_p.tile([4, b_c], BF16)

            # ---------- phase 0: load x/y, layernorm(y), transposes ----------
            # Per token-half: x transposes + gating first (qkv needs cb),
            # then ny. Gating softmax/top-2 is batched across the half's 4
            # tiles in wide DVE ops (small-op fixed costs dominate
            # otherwise); gating logits are O(3) so exp needs no max-sub.
            with ExitStack() as ctx_p0:
                xin = ctx_p0.enter_context(tc.tile_pool(name="xin", bufs=3))
                yin = ctx_p0.enter_context(tc.tile_pool(name="yin", bufs=3))
                nrm = ctx_p0.enter_context(tc.tile_pool(name="nrm", bufs=3))
                stat = ctx_p0.enter_context(tc.tile_pool(name="stat", bufs=2))
                gsmall = ctx_p0.enter_context(tc.tile_pool(name="gsm", bufs=2))
                xf = ctx_p0.enter_context(tc.tile_pool(name="xf", bufs=2))
                tp_ps = ctx_p0.enter_context(
                    tc.tile_pool(name="tp_ps", bufs=2, space="PSUM"))
                tpb_ps = ctx_p0.enter_context(
                    tc.tile_pool(name="tpb_ps", bufs=2, space="PSUM"))
                g_ps = ctx_p0.enter_context(
                    tc.tile_pool(name="g_ps", bufs=2, space="PSUM"))
                cr_ps = ctx_p0.enter_context(
                    tc.tile_pool(name="cr_ps", bufs=1, space="PSUM"))
                crows_ps = cr_ps.tile([4, b_c], F32)

                for hb in range(2):
                    tiles = range(hb * T2, (hb + 1) * T2)
                    # --- x loads, f32 transposes, gating matmuls ---
                    gps_all = g_ps.tile([128, T2, E], F32, tag="gpsa")
                    xts = []
                    for i, t in enumerate(tiles):
                        xt = xin.tile([128, DIM], F32, tag=f"xt{i}")
                        nc.sync.dma_start(out=xt, in_=x_r[t])
                        xts.append(xt)
                        xf_t = xf.tile([128, KD, 128], F32, tag="xf_t")
                        for grp in range(KD // 4):
                            pst = tp_ps.tile([128, 4, 128], F32, tag="tp")
                            for j in range(4):
                                kd = grp * 4 + j
                                nc.tensor.transpose(
                                    pst[:, j, :],
                                    xt[:, kd * 128:(kd + 1) * 128], ident_f)
                            gsl = slice(grp * 4, (grp + 1) * 4)
                            nc.scalar.copy(
                                out=xT[:, gsl, t * 128:(t + 1) * 128], in_=pst)
                            nc.scalar.copy(out=xf_t[:, gsl, :], in_=pst)
                        for kd in range(KD):
                            nc.tensor.matmul(gps_all[:, i, :], xf_t[:, kd, :],
                                             wg_sb[:, kd, :],
                                             start=(kd == 0),
                                             stop=(kd == KD - 1))
                    # --- batched gating softmax + top-2 over the half ---
                    bgb = (bg_sb.rearrange("p (o e) -> p o e", o=1)
                           .to_broadcast([128, T2, E]))
                    glog = gsmall.tile([128, T2, E], F32, tag="glog")
                    nc.vector.tensor_add(glog, gps_all, bgb)
                    gexp = gsmall.tile([128, T2, E], F32, tag="gexp")
                    nc.scalar.activation(out=gexp, in_=glog, func=AF.Exp,
                                         bias=zero_t, scale=1.0)
                    gden = gsmall.tile([128, T2, 1], F32, tag="gden")
                    nc.vector.reduce_sum(gden, gexp, AX.X)
                    grec = gsmall.tile([128, T2, 1], F32, tag="grec")
                    nc.vector.reciprocal(out=grec, in_=gden)
                    gate = gsmall.tile([128, T2, E], F32, tag="gate")
                    nc.vector.tensor_mul(gate, gexp,
                                         grec.to_broadcast([128, T2, E]))
                    m1 = gsmall.tile([128, T2, 1], F32, tag="m1")
                    nc.vector.tensor_reduce(out=m1, in_=gate, axis=AX.X,
                                            op=OP.max)
                    eq1 = gsmall.tile([128, T2, E], F32, tag="eq1")
                    nc.vector.tensor_tensor(out=eq1, in0=gate,
                                            in1=m1.to_broadcast([128, T2, E]),
                                            op=OP.is_equal)
                    msk = gsmall.tile([128, T2, E], F32, tag="msk")
                    nc.vector.scalar_tensor_tensor(out=msk, in0=eq1,
                                                   scalar=NEG_BIG, in1=gate,
                                                   op0=OP.mult, op1=OP.add)
                    m2 = gsmall.tile([128, T2, 1], F32, tag="m2")
                    nc.vector.tensor_reduce(out=m2, in_=msk, axis=AX.X,
                                            op=OP.max)
                    keep = gsmall.tile([128, T2, E], F32, tag="keep")
                    nc.vector.tensor_tensor(out=keep, in0=gate,
                                            in1=m2.to_broadcast([128, T2, E]),
                                            op=OP.is_ge)
                    c_all = gsmall.tile([128, T2, E], F32, tag="c_all")
                    nc.vector.tensor_mul(c_all, gate, keep)
                    for i, t in enumerate(tiles):
                        nc.tensor.transpose(
                            crows_ps[:, t * 128:(t + 1) * 128],
                            c_all[:, i, :], ident_f)
                    hsl = slice(hb * TH, (hb + 1) * TH)
                    nc.vector.tensor_copy(out=crows[:, hsl],
                                          in_=crows_ps[:, hsl])
                    nc.sync.dma_start(out=csc_d.ap()[:, hsl],
                                      in_=crows[:, hsl])
                    for e in range(E):
                        nc.sync.dma_start(
                            out=cb[:, e, hsl],
                            in_=csc_d.ap()[e:e + 1, hsl]
                            .to_broadcast([128, TH]))

                # second pass: layernorm(y) + bf16 transposes per half —
                # emitted after BOTH halves' gating so the half-1 gating
                # vector chain isn't queued behind half-0's LN on DVE
                for hb in range(2):
                    tiles = range(hb * T2, (hb + 1) * T2)
                    mv_all = stat.tile([128, T2, 2], F32, tag="mv_all")
                    yts0 = []
                    for i, t in enumerate(tiles):
                        yt = yin.tile([128, DIM], F32, tag=f"yt{i}")
                        nc.sync.dma_start(out=yt, in_=y_r[t])
                        yts0.append(yt)
                        st6 = stat.tile([128, 2, 6], F32, tag="st6")
                        yv = yt.rearrange("p (s d) -> p s d", s=2)
                        for s in range(2):
                            nc.vector.bn_stats(out=st6[:, s, :], in_=yv[:, s, :])
                        nc.vector.bn_aggr(out=mv_all[:, i, :], in_=st6)
                    sd = stat.tile([128, T2], F32, tag="sd")
                    nc.scalar.activation(out=sd, in_=mv_all[:, :, 1],
                                         func=AF.Sqrt, bias=eps_t, scale=1.0)
                    rstd = stat.tile([128, T2], F32, tag="rstd")
                    nc.vector.reciprocal(out=rstd, in_=sd)
                    for i, t in enumerate(tiles):
                        ny = nrm.tile([128, DIM], BF16, tag="ny")
                        nc.vector.tensor_scalar(out=ny, in0=yts0[i],
                                                scalar1=mv_all[:, i, 0:1],
                                                scalar2=rstd[:, i:i + 1],
                                                op0=OP.subtract, op1=OP.mult)
                        for grp in range(KD // 4):
                            pstb = tpb_ps.tile([128, 4, 128], BF16, tag="tpb")
                            for j in range(4):
                                kd = grp * 4 + j
                                nc.tensor.transpose(
                                    pstb[:, j, :],
                                    ny[:, kd * 128:(kd + 1) * 128], ident_bf)
                            gsl = slice(grp * 4, (grp + 1) * 4)
                            tsl128 = slice(t * 128, (t + 1) * 128)
                            nc.vector.tensor_copy(out=nyT[:, gsl, tsl128],
                                                  in_=pstb)
                            nc.scalar.copy(out=ny8[:, gsl, tsl128], in_=pstb)

            # ---------- phase 2: qkv fp8 DoubleRow over token halves ----------
            with ExitStack() as ctx_p2:
                wstr = ctx_p2.enter_context(tc.tile_pool(name="wstr", bufs=6))
                scl8 = ctx_p2.enter_context(tc.tile_pool(name="scl8", bufs=6))
                qk_ps = ctx_p2.enter_context(
                    tc.tile_pool(name="qk_ps", bufs=1, space="PSUM"))
                for th in range(2):
                    tsl = slice(th * TH, (th + 1) * TH)
                    ps = [qk_ps.tile([128, TH], F32, tag=f"qk{m}",
                                     name=f"qk{m}_{th}") for m in range(KD)]
                    step = 0
                    for e in range(E):
                        for k2 in range(KD2):
                            for which, w_d2 in enumerate((wq_d, wkv_d)):
                                wt8 = wstr.tile([128, 2, DIM], F8, tag="wt8")
                                nc.sync.dma_start(out=wt8, in_=w_d2.ap()[e, k2])
                                act = xT if which else nyT
                                seng = nc.gpsimd if which else nc.vector
                                cbb = (cb[:, e:e + 1, tsl]
                                       .to_broadcast([128, 2, TH]))
                                sc8 = scl8.tile([128, 2, TH], F8,
                                                tag=f"sc8{which}")
                                seng.tensor_mul(
                                    sc8, act[:, 2 * k2:2 * k2 + 2, tsl], cbb)
                                for m in range(KD):
                                    nc.tensor.matmul(
                                        ps[m],
                                        wt8[:, :, m * 128:(m + 1) * 128],
                                        sc8,
                                        start=(step == 0), stop=False,
                                        perf_mode=DR)
                                step += 1
                    # bias step: sum_e c[e,t] * bq[e,f] (bf16, normal mode)
                    for m in range(KD):
                        nc.tensor.matmul(ps[m], bq_sb[:, m, :], crows[:, tsl],
                                         start=False, stop=True)
                    for m in range(KD):
                        nc.vector.tensor_scalar_mul(sT[:, m, tsl], ps[m], WSI)

        # ---- phases 3-6. PE executes in program order, so emission order
        # IS the PE schedule: s-transposes, fc1, fc2 (needs only hT), THEN
        # the combine-dependent o-transposes, proj, final transposes. The
        # attention combine (vector) hides under fc1+fc2's PE time. ----
        with ExitStack() as ctxb:
            hT_p = ctxb.enter_context(tc.tile_pool(name="hT", bufs=1))
            hT = hT_p.tile([128, MH, b_c], F8 if FP8_FC2 else BF16)
            # at_ps/f1_ps stay open through phase 5 so f2_ps's 4 banks can
            # only alias qkv banks (readers long done) — NOT the transpose
            # banks whose last reader waits on the attention combine (that
            # WAR chain would serialize fc2 behind the combine)
            at_ps = ctxb.enter_context(
                tc.tile_pool(name="at_ps", bufs=2, space="PSUM"))
            f1_ps = ctxb.enter_context(
                tc.tile_pool(name="f1_ps", bufs=2, space="PSUM"))
            stok_p = ctxb.enter_context(tc.tile_pool(name="stok", bufs=1))
            asm = ctxb.enter_context(tc.tile_pool(name="asm", bufs=1))
            scr = ctxb.enter_context(tc.tile_pool(name="scr", bufs=1))
            oac_p = ctxb.enter_context(tc.tile_pool(name="oac", bufs=2))
            with ExitStack() as ctx_p3:
                w1str = ctx_p3.enter_context(tc.tile_pool(name="w1str", bufs=4))

                # s^T -> token-major s_all (all tiles)
                s_all = stok_p.tile([128, T, DIM], BF16)
                for t in range(T):
                    for grp in range(KD // 4):
                        pst = at_ps.tile([128, 4, 128], BF16, tag="atp")
                        for j in range(4):
                            mf = grp * 4 + j
                            nc.tensor.transpose(
                                pst[:, j, :],
                                sT[:, mf, t * 128:(t + 1) * 128], ident_bf)
                        nc.vector.tensor_copy(
                            out=s_all[:, t, grp * 512:(grp + 1) * 512],
                            in_=pst)

                # batched gram: for each head pair, big mul + reduce
                gram = asm.tile([128, T, H * H], F32)
                for h in range(H):
                    for g in range(h, H):
                        prod = scr.tile([128, T, HD], BF16, tag="prod")
                        nc.vector.tensor_mul(
                            prod, s_all[:, :, h * HD:(h + 1) * HD],
                            s_all[:, :, g * HD:(g + 1) * HD])
                        nc.vector.reduce_sum(
                            gram[:, :, h * H + g:h * H + g + 1], prod, AX.X)
                        if g != h:
                            nc.vector.tensor_copy(
                                out=gram[:, :, g * H + h:g * H + h + 1],
                                in_=gram[:, :, h * H + g:h * H + g + 1])
                # no-max softmax: gram*SCALE bounded (~13), exp is f32-safe
                pexp = asm.tile([128, T, H * H], F32)
                nc.scalar.activation(out=pexp, in_=gram, func=AF.Exp,
                                     bias=zero_t, scale=SCALE)
                den = asm.tile([128, T, H], F32)
                nc.vector.reduce_sum(
                    den, pexp.rearrange("p t (h g) -> p t h g", h=H), AX.X)
                rden = asm.tile([128, T, H], F32)
                nc.vector.reciprocal(out=rden, in_=den)
                an = asm.tile([128, T, H, H], BF16)
                nc.vector.tensor_mul(
                    an, pexp.rearrange("p t (h g) -> p t h g", h=H),
                    rden.rearrange("p t (h o) -> p t h o", o=1)
                    .to_broadcast([128, T, H, H]))

                # ---------- phase 4: MLP fc1 (gelu fused into eviction) ----
                if FP8_FC1:
                    for mh in range(MH):
                        w1t8 = w1str.tile([128, KD2, 2, 128], F8, tag="w1t8")
                        nc.sync.dma_start(
                            out=w1t8,
                            in_=w1_d.ap()[:, :, :, mh * 128:(mh + 1) * 128])
                        for ch in range(NCH):
                            psf = f1_ps.tile([128, CHUNK], F32, tag="psf")
                            for k2 in range(KD2):
                                nc.tensor.matmul(
                                    psf, w1t8[:, k2, :, :],
                                    ny8[:, 2 * k2:2 * k2 + 2,
                                        ch * CHUNK:(ch + 1) * CHUNK],
                                    start=(k2 == 0), stop=(k2 == KD2 - 1),
                                    perf_mode=DR)
                            nc.scalar.activation(
                                out=hT[:, mh, ch * CHUNK:(ch + 1) * CHUNK],
                                in_=psf, func=AF.Gelu,
                                bias=bm1_sb[:, mh:mh + 1], scale=WSI)
                else:
                    for mh in range(MH):
                        w1t = w1str.tile([128, KD, 128], BF16, tag="w1t")
                        nc.sync.dma_start(
                            out=w1t,
                            in_=w1_d.ap()[:, :, mh * 128:(mh + 1) * 128]
                            .rearrange("k p f -> p k f"))
                        for ch in range(NCH):
                            psf = f1_ps.tile([128, CHUNK], F32, tag="psf")
                            for kd in range(KD):
                                nc.tensor.matmul(
                                    psf, w1t[:, kd, :],
                                    ny8[:, kd, ch * CHUNK:(ch + 1) * CHUNK],
                                    start=(kd == 0), stop=(kd == KD - 1))
                            nc.scalar.activation(
                                out=hT[:, mh, ch * CHUNK:(ch + 1) * CHUNK],
                                in_=psf, func=AF.Gelu,
                                bias=bm1_sb[:, mh:mh + 1], scale=WSI)

            # ---------- phase 5: fc2, then combine+o-transposes, then proj
            # (two-stage accumulation so fc2's PE work runs during the
            # attention combine without any ordering/PSUM dependence) ----
            with ExitStack() as ctx_p5:
                w2str = ctx_p5.enter_context(tc.tile_pool(name="w2str", bufs=6))
                res2_p = ctx_p5.enter_context(tc.tile_pool(name="res2", bufs=1))
                outok_p = ctx_p5.enter_context(tc.tile_pool(name="outok", bufs=3))
                ytail = ctx_p5.enter_context(tc.tile_pool(name="ytail", bufs=1))
                f2_ps = ctx_p5.enter_context(
                    tc.tile_pool(name="f2_ps", bufs=1, space="PSUM"))
                res2 = res2_p.tile([128, KD, b_c], BF16)

                # prefetch residual-y tiles for phase 6
                yts = []
                for t in range(T):
                    yt2 = ytail.tile([128, DIM], F32, tag=f"yt{t}",
                                     name=f"yt2_{t}")
                    nc.sync.dma_start(out=yt2, in_=y_r[t])
                    yts.append(yt2)

                # --- stage A: fc2 standalone, evict partial (bias folded) ---
                for g2i in range(4):        # groups of 2 mf tiles
                    ps2 = [[f2_ps.tile([128, CHUNK], F32, tag=f"f2_{m}_{ch}",
                                       name=f"f2_{m}_{ch}_{g2i}")
                            for ch in range(NCH)] for m in range(2)]
                    cols = slice(g2i * 256, (g2i + 1) * 256)
                    for kh2 in range(MH2):
                        w2t8 = w2str.tile([128, 2, 256], F8, tag="w2t8")
                        nc.sync.dma_start(out=w2t8,
                                          in_=w2_d.ap()[kh2, :, :, cols])
                        for m in range(2):
                            for ch in range(NCH):
                                nc.tensor.matmul(
                                    ps2[m][ch],
                                    w2t8[:, :, m * 128:(m + 1) * 128],
                                    hT[:, 2 * kh2:2 * kh2 + 2,
                                       ch * CHUNK:(ch + 1) * CHUNK],
                                    start=(kh2 == 0),
                                    stop=(kh2 == MH2 - 1),
                                    perf_mode=DR)
                    for m in range(2):
                        mf = g2i * 2 + m
                        for ch in range(NCH):
                            nc.scalar.activation(
                                out=res2[:, mf, ch * CHUNK:(ch + 1) * CHUNK],
                                in_=ps2[m][ch], func=AF.Identity,
                                bias=bpb2_sb[:, mf:mf + 1], scale=WSI)

                # --- stage B: attention combine + o -> oT transposes ---
                # (emitted after fc2 so these combine-dependent PE ops queue
                # behind fc2's matmuls, not in front of them)
                for th2 in range(2):
                    t2sl = slice(th2 * T2, (th2 + 1) * T2)
                    o_acc = oac_p.tile([128, T2, DIM], BF16, tag="oacc")
                    ov = o_acc.rearrange("p t (h d) -> p t h d", h=H)
                    for g in range(H):
                        sgb = (s_all[:, t2sl, g * HD:(g + 1) * HD]
                               .rearrange("p t (o d) -> p t o d", o=1)
                               .to_broadcast([128, T2, H, HD]))
                        agb = (an[:, t2sl, :, g:g + 1]
                               .to_broadcast([128, T2, H, HD]))
                        if g == 0:
                            nc.vector.tensor_mul(ov, sgb, agb)
                        else:
                            tmp = scr.tile([128, T2, H, HD], BF16, tag="ctmp")
                            nc.vector.tensor_mul(tmp, sgb, agb)
                            nc.vector.tensor_add(ov, ov, tmp)
                    for i in range(T2):
                        t = th2 * T2 + i
                        for grp in range(KD // 4):
                            pst = at_ps.tile([128, 4, 128], BF16, tag="atp")
                            for j in range(4):
                                kd = grp * 4 + j
                                nc.tensor.transpose(
                                    pst[:, j, :],
                                    o_acc[:, i, kd * 128:(kd + 1) * 128],
                                    ident_bf)
                            nc.vector.tensor_copy(
                                out=oT[:, grp * 4:(grp + 1) * 4,
                                       t * 128:(t + 1) * 128],
                                in_=pst)

                # --- stage C: proj, merged into res2 on eviction ---
                for g2i in range(4):
                    psp = [[f2_ps.tile([128, CHUNK], F32, tag=f"f2_{m}_{ch}",
                                       name=f"pj_{m}_{ch}_{g2i}")
                            for ch in range(NCH)] for m in range(2)]
                    cols = slice(g2i * 256, (g2i + 1) * 256)
                    for kd in range(KD):
                        wpt = w2str.tile([128, 256], BF16, tag="wpt")
                        nc.sync.dma_start(out=wpt, in_=wp_d.ap()[kd, :, cols])
                        for m in range(2):
                            for ch in range(NCH):
                                nc.tensor.matmul(
                                    psp[m][ch], wpt[:, m * 128:(m + 1) * 128],
                                    oT[:, kd, ch * CHUNK:(ch + 1) * CHUNK],
                                    start=(kd == 0), stop=(kd == KD - 1))
                    for m in range(2):
                        mf = g2i * 2 + m
                        for ch in range(NCH):
                            csl = slice(ch * CHUNK, (ch + 1) * CHUNK)
                            nc.vector.scalar_tensor_tensor(
                                out=res2[:, mf, csl], in0=psp[m][ch],
                                scalar=WSI, in1=res2[:, mf, csl],
                                op0=OP.mult, op1=OP.add)

                # ---------- phase 6: final transpose + residual + store ----
                for t in range(T):
                    yt2 = yts[t]
                    out_tok = outok_p.tile([128, DIM], F32, tag="out_tok")
                    for grp in range(KD // 4):
                        pst = at_ps.tile([128, 4, 128], BF16, tag="atp")
                        for j in range(4):
                            mf = grp * 4 + j
                            nc.tensor.transpose(
                                pst[:, j, :],
                                res2[:, mf, t * 128:(t + 1) * 128], ident_bf)
                        cols = slice(grp * 512, (grp + 1) * 512)
                        nc.vector.tensor_tensor(
                            out=out_tok[:, cols], in0=pst,
                            in1=yt2[:, cols], op=OP.add)
                    nc.sync.dma_start(out=out_r[t], in_=out_tok)

    return nc


MAX_WAITS = 1


def split_big_waits(nc, limit=MAX_WAITS):
    """Walrus rejects instructions carrying too many sem waits; move the
    overflow onto preceding single-wait NoOps on the same engine."""
    n = 0
    for fn in nc.m.functions:
        for blk in fn.blocks:
            new_insts = []
            for inst in blk.instructions:
                si = inst.sync_info
                if si is not None and len(si.on_wait) > limit:
                    waits = list(si.on_wait)
                    while len(waits) > limit:
                        w, waits = waits[0], waits[1:]
                        nop = mybir.InstNoOp(name=f"WSPLIT-{nc.next_id()}")
                        nop.engine = inst.engine
                        nop.sync_info = mybir.SyncInfo(on_wait=[w], on_update=[])
                        new_insts.append(nop)
                        n += 1
                    si.on_wait = waits
                new_insts.append(inst)
            blk.instructions[:] = new_insts
    return n


_NC_CACHE = {}


def get_nc(b_c=B_C):
    """Build + apply the walrus wait-split workaround (HW compile path)."""
    if b_c not in _NC_CACHE:
        nc = build_kernel(b_c)
        split_big_waits(nc)
        _NC_CACHE[b_c] = nc
    return _NC_CACHE[b_c]


def make_in_maps(inputs, b_c=B_C, ncores=NCORES):
    w = prep_weights(
        np.asarray(inputs["Wg"]), np.asarray(inputs["bg"]),
        np.asarray(inputs["Wqkv"]), np.asarray(inputs["Wp"]),
        np.asarray(inputs["bp"]), np.asarray(inputs["g1"]),
        np.asarray(inputs["bn1"]), np.asarray(inputs["g2"]),
        np.asarray(inputs["bn2"]), np.asarray(inputs["W1"]),
        np.asarray(inputs["bm1"]), np.asarray(inputs["W2"]),
        np.asarray(inputs["bm2"]))
    x = f32(np.asarray(inputs["x"]))
    y = f32(np.asarray(inputs["y"]))
    in_maps = []
    for c in range(ncores):
        sl = slice(c * b_c, (c + 1) * b_c)
        in_maps.append({"x": x[sl], "y": y[sl], **w})
    return in_maps


def kernel(**inputs):
    nc = get_nc(B_C)
    in_maps = make_in_maps(inputs)
    res = run_bass_kernel_spmd(nc, in_maps, core_ids=list(range(NCORES)))
    return np.concatenate([res.results[c]["out"] for c in range(NCORES)], axis=0)


# revision 48
# speedup vs baseline: 1.1073x; 1.0408x over previous
"""Trainium2 Bass kernel for nn_Decoder_48052094107929 (moe_routing).

Data-parallel over 8 NeuronCores: batch B=8192 split into 8 shards of 1024
tokens; all weights replicated. Per core the whole decoder block runs as one
Tile kernel:

  phase 0: load x/y, layernorm(y) (token-major), PE-transpose x and ny to
           feature-major; gating softmax + top-2 -> c; broadcast c -> cb
  phase 2: qkv s = sum_e Wq'[e]@(c_e*ny) + Wkv'[e]@(c_e*x) + c@bq, fp8-e4m3
           DoubleRow matmuls over token halves (8 PSUM banks each);
           c-scaling muls run bf16 on DVE (2x/4x modes need all-2-byte
           operands) with the fp8 cast on the scalar engine (ny side) or
           directly on gpsimd (x side)
  phase 3: attention, batched across all 8 token tiles: gram via big DVE
           mul+reduce ops, no-max softmax (gram*SCALE <= ~13), combine via
           broadcast-AP muls; o transposed back to feature-major
  phase 4: fc1 fp8 DoubleRow, gelu fused into eviction
  phase 5: proj (bf16 - precision) + fc2 (fp8) share one PSUM accumulation
  phase 6: transpose + residual y + store

All big-matmul weights are pre-scaled x256 host-side (dodges fp8-e4m3
subnormals; exact in bf16) and every PSUM eviction applies scale 1/256.
LayerNorm gains/biases folded into weights host-side.
"""

import numpy as np
import ml_dtypes

import concourse.bass as bass
import concourse.mybir as mybir
import concourse.tile as tile
from concourse.bass_utils import run_bass_kernel_spmd
from concourse.masks import make_identity

# ---- problem constants (hardcoded per harness contract) ----
B = 8192
DIM = 1024
E = 4
H = 4
TOPK = 2
HD = DIM // H          # 256
SCALE = HD ** -0.5
HID = 4 * DIM          # 4096
EPS = 1e-5
NCORES = 8
B_C = B // NCORES      # 1024 tokens per core

F32 = mybir.dt.float32
BF16 = mybir.dt.bfloat16
F8 = mybir.dt.float8e4
AX = mybir.AxisListType
OP = mybir.AluOpType
AF = mybir.ActivationFunctionType
DR = mybir.MatmulPerfMode.DoubleRow

KD = DIM // 128        # 8  d-tiles
KD2 = KD // 2          # 4  d-pair-tiles
MH = HID // 128        # 32 hidden tiles
MH2 = MH // 2          # 16 hidden pair-tiles
NEG_BIG = -1.0e30
WS = 256.0             # weight pre-scale (fp8 subnormal dodge)
WSI = 1.0 / WS

FP8_FC1 = True
FP8_FC2 = True


def bf(a):
    return np.ascontiguousarray(a.astype(ml_dtypes.bfloat16))


def f32(a):
    return np.ascontiguousarray(a.astype(np.float32))


def f8(a):
    return np.ascontiguousarray(a.astype(ml_dtypes.float8_e4m3))


def pair8(wT, nf):
    """[d, f] weight (d=contraction) -> fp8 DoubleRow layout
    [KD2, 128, 2, nf], scaled by WS."""
    k2 = wT.shape[0] // 256
    return f8((wT * WS).reshape(k2, 2, 128, nf).transpose(0, 2, 1, 3))


def prep_weights(Wg, bg, Wqkv, Wp, bp, g1, bn1, g2, bn2, W1, bm1, W2, bm2):
    """Host-side, input-independent weight layout transforms."""
    Wq = Wqkv[:, :DIM, :]                        # [E, DIM, DIM] (f, d)
    Wk = Wqkv[:, DIM:2 * DIM, :]
    Wv = Wqkv[:, 2 * DIM:, :]
    Wqp = Wq * g1[None, None, :]                 # fold norm1 gamma into cols
    bq = np.einsum("efd,d->ef", Wq, bn1)         # [E, DIM] bias from norm1 beta
    Wkvs = Wk + Wv                               # aliasing bug: k+v share weights

    W1p = W1 * g2[None, :]
    bm1p = bm1 + W1 @ bn2

    out = {
        "wg": f32(Wg.T.reshape(KD, 128, E)),
        "bgv": f32(bg.reshape(1, E)),
        "bm1v": f32(bm1p.reshape(MH, 128).T),              # [128,32]
        "bpb2": f32((bp + bm2).reshape(KD, 128).T),        # [128,8]
        "bq": bf((bq * WS).reshape(E, KD, 128)),
        # qkv fp8 DoubleRow weights
        "wq8": np.stack([pair8(Wqp[e].T, DIM) for e in range(E)]),
        "wkv8": np.stack([pair8(Wkvs[e].T, DIM) for e in range(E)]),
        # proj always bf16 (largest per-FLOP error contributor)
        "wp": bf(Wp.T.reshape(KD, 128, DIM) * WS),
    }
    if FP8_FC1:
        out["w18"] = np.ascontiguousarray(
            pair8(W1p.T, HID).transpose(1, 0, 2, 3))       # [128,4,2,4096]
    else:
        out["w1"] = bf(W1p.T.reshape(KD, 128, HID) * WS)
    if FP8_FC2:
        out["w28"] = pair8(W2.T, DIM)                      # [16,128,2,1024]
    else:
        out["w2"] = bf(W2.T.reshape(MH, 128, DIM) * WS)
    return out


def build_kernel(b_c=B_C):
    """Build the Bass module for one core processing b_c tokens."""
    nc = bass.Bass("TRN2", target_bir_lowering=False, debug=False)

    T = b_c // 128                 # token tiles
    TH = b_c // 2                  # tokens per half
    CHUNK = min(512, b_c)
    NCH = b_c // CHUNK
    T2 = T // 2

    # ---- DRAM tensors ----
    x_d = nc.dram_tensor("x", [b_c, DIM], F32, kind="ExternalInput")
    y_d = nc.dram_tensor("y", [b_c, DIM], F32, kind="ExternalInput")
    wq_d = nc.dram_tensor("wq8", [E, KD2, 128, 2, DIM], F8, kind="ExternalInput")
    wkv_d = nc.dram_tensor("wkv8", [E, KD2, 128, 2, DIM], F8,
                           kind="ExternalInput")
    wp_d = nc.dram_tensor("wp", [KD, 128, DIM], BF16, kind="ExternalInput")
    if FP8_FC1:
        w1_d = nc.dram_tensor("w18", [128, KD2, 2, HID], F8,
                              kind="ExternalInput")
    else:
        w1_d = nc.dram_tensor("w1", [KD, 128, HID], BF16, kind="ExternalInput")
    if FP8_FC2:
        w2_d = nc.dram_tensor("w28", [MH2, 128, 2, DIM], F8,
                              kind="ExternalInput")
    else:
        w2_d = nc.dram_tensor("w2", [MH, 128, DIM], BF16, kind="ExternalInput")
    wg_d = nc.dram_tensor("wg", [KD, 128, E], F32, kind="ExternalInput")
    bq_d = nc.dram_tensor("bq", [E, KD, 128], BF16, kind="ExternalInput")
    bg_d = nc.dram_tensor("bgv", [1, E], F32, kind="ExternalInput")
    bm1_d = nc.dram_tensor("bm1v", [128, MH], F32, kind="ExternalInput")
    bpb2_d = nc.dram_tensor("bpb2", [128, KD], F32, kind="ExternalInput")
    out_d = nc.dram_tensor("out", [b_c, DIM], F32, kind="ExternalOutput")
    csc_d = nc.dram_tensor("cscratch", [E, b_c], BF16, kind="Internal")

    x_r = x_d.ap().rearrange("(t p) d -> t p d", p=128)
    y_r = y_d.ap().rearrange("(t p) d -> t p d", p=128)
    out_r = out_d.ap().rearrange("(t p) d -> t p d", p=128)

    from contextlib import ExitStack

    with tile.TileContext(nc) as tc, ExitStack() as ctx0:
        consts = ctx0.enter_context(tc.tile_pool(name="consts", bufs=1))
        ident_bf = consts.tile([128, 128], BF16)
        make_identity(nc, ident_bf)
        ident_f = consts.tile([128, 128], F32)
        make_identity(nc, ident_f)
        eps_t = consts.tile([128, 1], F32)
        nc.vector.memset(eps_t, EPS)
        zero_t = consts.tile([128, 1], F32)
        nc.vector.memset(zero_t, 0.0)
        bg_sb = consts.tile([128, E], F32)
        nc.sync.dma_start(out=bg_sb, in_=bg_d.ap().to_broadcast([128, E]))
        wg_sb = consts.tile([128, KD, E], F32)
        nc.sync.dma_start(out=wg_sb, in_=wg_d.ap().rearrange("k p e -> p k e"))
        bq_sb = consts.tile([4, KD, 128], BF16)
        nc.sync.dma_start(out=bq_sb, in_=bq_d.ap())
        bm1_sb = consts.tile([128, MH], F32)
        nc.sync.dma_start(out=bm1_sb, in_=bm1_d.ap())
        bpb2_sb = consts.tile([128, KD], F32)
        nc.sync.dma_start(out=bpb2_sb, in_=bpb2_d.ap())

        ny8_p = ctx0.enter_context(tc.tile_pool(name="ny8", bufs=1))
        ny8 = ny8_p.tile([128, KD, b_c], F8 if FP8_FC1 else BF16)
        oT_p = ctx0.enter_context(tc.tile_pool(name="oT", bufs=1))
        oT = oT_p.tile([128, KD, b_c], BF16)
        sT_p = ctx0.enter_context(tc.tile_pool(name="sT", bufs=1))
        sT = sT_p.tile([128, KD, b_c], BF16)

        with ExitStack() as ctxa:
            nyT_p = ctxa.enter_context(tc.tile_pool(name="nyT", bufs=1))
            nyT = nyT_p.tile([128, KD, b_c], BF16)
            xT_p = ctxa.enter_context(tc.tile_pool(name="xT", bufs=1))
            xT = xT_p.tile([128, KD, b_c], BF16)
            cb_p = ctxa.enter_context(tc.tile_pool(name="cb", bufs=1))
            cb = cb_p.tile([128, E, b_c], BF16)
            crows_p = ctxa.enter_context(tc.tile_pool(name="crows", bufs=1))
            crows = crows_p.tile([4, b_c], BF16)

            # ---------- phase 0: load x/y, layernorm(y), transposes ----------
            # Per token-half: x transposes + gating first (qkv needs cb),
            # then ny. Gating softmax/top-2 is batched across the half's 4
            # tiles in wide DVE ops (small-op fixed costs dominate
            # otherwise); gating logits are O(3) so exp needs no max-sub.
            with ExitStack() as ctx_p0:
                xin = ctx_p0.enter_context(tc.tile_pool(name="xin", bufs=3))
                yin = ctx_p0.enter_context(tc.tile_pool(name="yin", bufs=3))
                nrm = ctx_p0.enter_context(tc.tile_pool(name="nrm", bufs=3))
                stat = ctx_p0.enter_context(tc.tile_pool(name="stat", bufs=2))
                gsmall = ctx_p0.enter_context(tc.tile_pool(name="gsm", bufs=2))
                xf = ctx_p0.enter_context(tc.tile_pool(name="xf", bufs=2))
                tp_ps = ctx_p0.enter_context(
                    tc.tile_pool(name="tp_ps", bufs=2, space="PSUM"))
                tpb_ps = ctx_p0.enter_context(
                    tc.tile_pool(name="tpb_ps", bufs=2, space="PSUM"))
                g_ps = ctx_p0.enter_context(
                    tc.tile_pool(name="g_ps", bufs=2, space="PSUM"))
                cr_ps = ctx_p0.enter_context(
                    tc.tile_pool(name="cr_ps", bufs=1, space="PSUM"))
                crows_ps = cr_ps.tile([4, b_c], F32)

                for hb in range(2):
                    tiles = range(hb * T2, (hb + 1) * T2)
                    # --- x loads, f32 transposes, gating matmuls ---
                    gps_all = g_ps.tile([128, T2, E], F32, tag="gpsa")
                    xts = []
                    for i, t in enumerate(tiles):
                        xt = xin.tile([128, DIM], F32, tag=f"xt{i}")
                        nc.sync.dma_start(out=xt, in_=x_r[t])
                        xts.append(xt)
                        xf_t = xf.tile([128, KD, 128], F32, tag="xf_t")
                        for grp in range(KD // 4):
                            pst = tp_ps.tile([128, 4, 128], F32, tag="tp")
                            for j in range(4):
                                kd = grp * 4 + j
                                nc.tensor.transpose(
                                    pst[:, j, :],
                                    xt[:, kd * 128:(kd + 1) * 128], ident_f)
                            gsl = slice(grp * 4, (grp + 1) * 4)
                            nc.scalar.copy(
                                out=xT[:, gsl, t * 128:(t + 1) * 128], in_=pst)
                            nc.scalar.copy(out=xf_t[:, gsl, :], in_=pst)
                        for kd in range(KD):
                            nc.tensor.matmul(gps_all[:, i, :], xf_t[:, kd, :],
                                             wg_sb[:, kd, :],
                                             start=(kd == 0),
                                             stop=(kd == KD - 1))
                    # --- batched gating softmax + top-2 over the half ---
                    bgb = (bg_sb.rearrange("p (o e) -> p o e", o=1)
                           .to_broadcast([128, T2, E]))
                    glog = gsmall.tile([128, T2, E], F32, tag="glog")
                    nc.vector.tensor_add(glog, gps_all, bgb)
                    gexp = gsmall.tile([128, T2, E], F32, tag="gexp")
                    nc.scalar.activation(out=gexp, in_=glog, func=AF.Exp,
                                         bias=zero_t, scale=1.0)
                    gden = gsmall.tile([128, T2, 1], F32, tag="gden")
                    nc.vector.reduce_sum(gden, gexp, AX.X)
                    grec = gsmall.tile([128, T2, 1], F32, tag="grec")
                    nc.vector.reciprocal(out=grec, in_=gden)
                    gate = gsmall.tile([128, T2, E], F32, tag="gate")
                    nc.vector.tensor_mul(gate, gexp,
                                         grec.to_broadcast([128, T2, E]))
                    m1 = gsmall.tile([128, T2, 1], F32, tag="m1")
                    nc.vector.tensor_reduce(out=m1, in_=gate, axis=AX.X,
                                            op=OP.max)
                    eq1 = gsmall.tile([128, T2, E], F32, tag="eq1")
                    nc.vector.tensor_tensor(out=eq1, in0=gate,
                                            in1=m1.to_broadcast([128, T2, E]),
                                            op=OP.is_equal)
                    msk = gsmall.tile([128, T2, E], F32, tag="msk")
                    nc.vector.scalar_tensor_tensor(out=msk, in0=eq1,
                                                   scalar=NEG_BIG, in1=gate,
                                                   op0=OP.mult, op1=OP.add)
                    m2 = gsmall.tile([128, T2, 1], F32, tag="m2")
                    nc.vector.tensor_reduce(out=m2, in_=msk, axis=AX.X,
                                            op=OP.max)
                    keep = gsmall.tile([128, T2, E], F32, tag="keep")
                    nc.vector.tensor_tensor(out=keep, in0=gate,
                                            in1=m2.to_broadcast([128, T2, E]),
                                            op=OP.is_ge)
                    c_all = gsmall.tile([128, T2, E], F32, tag="c_all")
                    nc.vector.tensor_mul(c_all, gate, keep)
                    for i, t in enumerate(tiles):
                        nc.tensor.transpose(
                            crows_ps[:, t * 128:(t + 1) * 128],
                            c_all[:, i, :], ident_f)
                    hsl = slice(hb * TH, (hb + 1) * TH)
                    nc.vector.tensor_copy(out=crows[:, hsl],
                                          in_=crows_ps[:, hsl])
                    nc.sync.dma_start(out=csc_d.ap()[:, hsl],
                                      in_=crows[:, hsl])
                    for e in range(E):
                        nc.sync.dma_start(
                            out=cb[:, e, hsl],
                            in_=csc_d.ap()[e:e + 1, hsl]
                            .to_broadcast([128, TH]))

                # second pass: layernorm(y) + bf16 transposes per half —
                # emitted after BOTH halves' gating so the half-1 gating
                # vector chain isn't queued behind half-0's LN on DVE
                for hb in range(2):
                    tiles = range(hb * T2, (hb + 1) * T2)
                    mv_all = stat.tile([128, T2, 2], F32, tag="mv_all")
                    yts0 = []
                    for i, t in enumerate(tiles):
                        yt = yin.tile([128, DIM], F32, tag=f"yt{i}")
                        nc.sync.dma_start(out=yt, in_=y_r[t])
                        yts0.append(yt)
                        st6 = stat.tile([128, 2, 6], F32, tag="st6")
                        yv = yt.rearrange("p (s d) -> p s d", s=2)
                        for s in range(2):
                            nc.vector.bn_stats(out=st6[:, s, :], in_=yv[:, s, :])
                        nc.vector.bn_aggr(out=mv_all[:, i, :], in_=st6)
                    sd = stat.tile([128, T2], F32, tag="sd")
                    nc.scalar.activation(out=sd, in_=mv_all[:, :, 1],
                                         func=AF.Sqrt, bias=eps_t, scale=1.0)
                    rstd = stat.tile([128, T2], F32, tag="rstd")
                    nc.vector.reciprocal(out=rstd, in_=sd)
                    for i, t in enumerate(tiles):
                        ny = nrm.tile([128, DIM], BF16, tag="ny")
                        nc.vector.tensor_scalar(out=ny, in0=yts0[i],
                                                scalar1=mv_all[:, i, 0:1],
                                                scalar2=rstd[:, i:i + 1],
                                                op0=OP.subtract, op1=OP.mult)
                        for grp in range(KD // 4):
                            pstb = tpb_ps.tile([128, 4, 128], BF16, tag="tpb")
                            for j in range(4):
                                kd = grp * 4 + j
                                nc.tensor.transpose(
                                    pstb[:, j, :],
                                    ny[:, kd * 128:(kd + 1) * 128], ident_bf)
                            gsl = slice(grp * 4, (grp + 1) * 4)
                            tsl128 = slice(t * 128, (t + 1) * 128)
                            nc.vector.tensor_copy(out=nyT[:, gsl, tsl128],
                                                  in_=pstb)
                            nc.scalar.copy(out=ny8[:, gsl, tsl128], in_=pstb)

            # ---------- phase 2: qkv fp8 DoubleRow over token halves ----------
            with ExitStack() as ctx_p2:
                wstr = ctx_p2.enter_context(tc.tile_pool(name="wstr", bufs=6))
                scl8 = ctx_p2.enter_context(tc.tile_pool(name="scl8", bufs=6))
                qk_ps = ctx_p2.enter_context(
                    tc.tile_pool(name="qk_ps", bufs=1, space="PSUM"))
                for th in range(2):
                    tsl = slice(th * TH, (th + 1) * TH)
                    ps = [qk_ps.tile([128, TH], F32, tag=f"qk{m}",
                                     name=f"qk{m}_{th}") for m in range(KD)]
                    step = 0
                    for e in range(E):
                        for k2 in range(KD2):
                            for which, w_d2 in enumerate((wq_d, wkv_d)):
                                wt8 = wstr.tile([128, 2, DIM], F8, tag="wt8")
                                nc.sync.dma_start(out=wt8, in_=w_d2.ap()[e, k2])
                                act = xT if which else nyT
                                seng = nc.gpsimd if which else nc.vector
                                cbb = (cb[:, e:e + 1, tsl]
                                       .to_broadcast([128, 2, TH]))
                                sc8 = scl8.tile([128, 2, TH], F8,
                                                tag=f"sc8{which}")
                                seng.tensor_mul(
                                    sc8, act[:, 2 * k2:2 * k2 + 2, tsl], cbb)
                                for m in range(KD):
                                    nc.tensor.matmul(
                                        ps[m],
                                        wt8[:, :, m * 128:(m + 1) * 128],
                                        sc8,
                                        start=(step == 0), stop=False,
                                        perf_mode=DR)
                                step += 1
                    # bias step: sum_e c[e,t] * bq[e,f] (bf16, normal mode)
                    for m in range(KD):
                        nc.tensor.matmul(ps[m], bq_sb[:, m, :], crows[:, tsl],
                                         start=False, stop=True)
                    for m in range(KD):
                        nc.scalar.activation(out=sT[:, m, tsl], in_=ps[m],
                                             func=AF.Identity, bias=zero_t,
                                             scale=WSI)

        # ---- phases 3-6. PE executes in program order, so emission order
        # IS the PE schedule: s-transposes, fc1, fc2 (needs only hT), THEN
        # the combine-dependent o-transposes, proj, final transposes. The
        # attention combine (vector) hides under fc1+fc2's PE time. ----
        with ExitStack() as ctxb:
            hT_p = ctxb.enter_context(tc.tile_pool(name="hT", bufs=1))
            hT = hT_p.tile([128, MH, b_c], F8 if FP8_FC2 else BF16)
            # at_ps/f1_ps stay open through phase 5 so f2_ps's 4 banks can
            # only alias qkv banks (readers long done) — NOT the transpose
            # banks whose last reader waits on the attention combine (that
            # WAR chain would serialize fc2 behind the combine)
            at_ps = ctxb.enter_context(
                tc.tile_pool(name="at_ps", bufs=2, space="PSUM"))
            f1_ps = ctxb.enter_context(
                tc.tile_pool(name="f1_ps", bufs=2, space="PSUM"))
            stok_p = ctxb.enter_context(tc.tile_pool(name="stok", bufs=1))
            asm = ctxb.enter_context(tc.tile_pool(name="asm", bufs=1))
            scr = ctxb.enter_context(tc.tile_pool(name="scr", bufs=2))
            oac_p = ctxb.enter_context(tc.tile_pool(name="oac", bufs=2))
            with ExitStack() as ctx_p3:
                w1str = ctx_p3.enter_context(tc.tile_pool(name="w1str", bufs=4))

                # s^T -> token-major s_all (all tiles)
                s_all = stok_p.tile([128, T, DIM], BF16)
                for t in range(T):
                    for grp in range(KD // 4):
                        pst = at_ps.tile([128, 4, 128], BF16, tag="atp")
                        for j in range(4):
                            mf = grp * 4 + j
                            nc.tensor.transpose(
                                pst[:, j, :],
                                sT[:, mf, t * 128:(t + 1) * 128], ident_bf)
                        nc.vector.tensor_copy(
                            out=s_all[:, t, grp * 512:(grp + 1) * 512],
                            in_=pst)

                # batched gram: for each head pair, big mul + reduce
                gram = asm.tile([128, T, H * H], F32)
                for h in range(H):
                    for g in range(h, H):
                        prod = scr.tile([128, T, HD], BF16, tag="prod")
                        nc.vector.tensor_mul(
                            prod, s_all[:, :, h * HD:(h + 1) * HD],
                            s_all[:, :, g * HD:(g + 1) * HD])
                        nc.vector.reduce_sum(
                            gram[:, :, h * H + g:h * H + g + 1], prod, AX.X)
                        if g != h:
                            nc.vector.tensor_copy(
                                out=gram[:, :, g * H + h:g * H + h + 1],
                                in_=gram[:, :, h * H + g:h * H + g + 1])
                # no-max softmax: gram*SCALE bounded (~13), exp is f32-safe
                pexp = asm.tile([128, T, H * H], F32)
                nc.scalar.activation(out=pexp, in_=gram, func=AF.Exp,
                                     bias=zero_t, scale=SCALE)
                den = asm.tile([128, T, H], F32)
                nc.vector.reduce_sum(
                    den, pexp.rearrange("p t (h g) -> p t h g", h=H), AX.X)
                rden = asm.tile([128, T, H], F32)
                nc.vector.reciprocal(out=rden, in_=den)
                an = asm.tile([128, T, H, H], BF16)
                nc.vector.tensor_mul(
                    an, pexp.rearrange("p t (h g) -> p t h g", h=H),
                    rden.rearrange("p t (h o) -> p t h o", o=1)
                    .to_broadcast([128, T, H, H]))

                # ---------- phase 4: MLP fc1 (gelu fused into eviction) ----
                if FP8_FC1:
                    for mh in range(MH):
                        w1t8 = w1str.tile([128, KD2, 2, 128], F8, tag="w1t8")
                        nc.sync.dma_start(
                            out=w1t8,
                            in_=w1_d.ap()[:, :, :, mh * 128:(mh + 1) * 128])
                        for ch in range(NCH):
                            psf = f1_ps.tile([128, CHUNK], F32, tag="psf")
                            for k2 in range(KD2):
                                nc.tensor.matmul(
                                    psf, w1t8[:, k2, :, :],
                                    ny8[:, 2 * k2:2 * k2 + 2,
                                        ch * CHUNK:(ch + 1) * CHUNK],
                                    start=(k2 == 0), stop=(k2 == KD2 - 1),
                                    perf_mode=DR)
                            nc.scalar.activation(
                                out=hT[:, mh, ch * CHUNK:(ch + 1) * CHUNK],
                                in_=psf, func=AF.Gelu,
                                bias=bm1_sb[:, mh:mh + 1], scale=WSI)
                else:
                    for mh in range(MH):
                        w1t = w1str.tile([128, KD, 128], BF16, tag="w1t")
                        nc.sync.dma_start(
                            out=w1t,
                            in_=w1_d.ap()[:, :, mh * 128:(mh + 1) * 128]
                            .rearrange("k p f -> p k f"))
                        for ch in range(NCH):
                            psf = f1_ps.tile([128, CHUNK], F32, tag="psf")
                            for kd in range(KD):
                                nc.tensor.matmul(
                                    psf, w1t[:, kd, :],
                                    ny8[:, kd, ch * CHUNK:(ch + 1) * CHUNK],
                                    start=(kd == 0), stop=(kd == KD - 1))
                            nc.scalar.activation(
                                out=hT[:, mh, ch * CHUNK:(ch + 1) * CHUNK],
                                in_=psf, func=AF.Gelu,
                                bias=bm1_sb[:, mh:mh + 1], scale=WSI)

            # ---------- phase 5: fc2, then combine+o-transposes, then proj
            # (two-stage accumulation so fc2's PE work runs during the
            # attention combine without any ordering/PSUM dependence) ----
            with ExitStack() as ctx_p5:
                w2str = ctx_p5.enter_context(tc.tile_pool(name="w2str", bufs=6))
                res2_p = ctx_p5.enter_context(tc.tile_pool(name="res2", bufs=1))
                outok_p = ctx_p5.enter_context(tc.tile_pool(name="outok", bufs=3))
                ytail = ctx_p5.enter_context(tc.tile_pool(name="ytail", bufs=1))
                f2_ps = ctx_p5.enter_context(
                    tc.tile_pool(name="f2_ps", bufs=1, space="PSUM"))
                res2 = res2_p.tile([128, KD, b_c], BF16)

                # prefetch residual-y tiles for phase 6
                yts = []
                for t in range(T):
                    yt2 = ytail.tile([128, DIM], F32, tag=f"yt{t}",
                                     name=f"yt2_{t}")
                    nc.sync.dma_start(out=yt2, in_=y_r[t])
                    yts.append(yt2)

                # --- stage A: fc2 standalone, evict partial (bias folded) ---
                for g2i in range(4):        # groups of 2 mf tiles
                    ps2 = [[f2_ps.tile([128, CHUNK], F32, tag=f"f2_{m}_{ch}",
                                       name=f"f2_{m}_{ch}_{g2i}")
                            for ch in range(NCH)] for m in range(2)]
                    cols = slice(g2i * 256, (g2i + 1) * 256)
                    for kh2 in range(MH2):
                        w2t8 = w2str.tile([128, 2, 256], F8, tag="w2t8")
                        nc.sync.dma_start(out=w2t8,
                                          in_=w2_d.ap()[kh2, :, :, cols])
                        for m in range(2):
                            for ch in range(NCH):
                                nc.tensor.matmul(
                                    ps2[m][ch],
                                    w2t8[:, :, m * 128:(m + 1) * 128],
                                    hT[:, 2 * kh2:2 * kh2 + 2,
                                       ch * CHUNK:(ch + 1) * CHUNK],
                                    start=(kh2 == 0),
                                    stop=(kh2 == MH2 - 1),
                                    perf_mode=DR)
                    for m in range(2):
                        mf = g2i * 2 + m
                        for ch in range(NCH):
                            nc.scalar.activation(
                                out=res2[:, mf, ch * CHUNK:(ch + 1) * CHUNK],
                                in_=ps2[m][ch], func=AF.Identity,
                                bias=bpb2_sb[:, mf:mf + 1], scale=WSI)

                # --- stage B: attention combine + o -> oT transposes ---
                # (emitted after fc2 so these combine-dependent PE ops queue
                # behind fc2's matmuls, not in front of them)
                for th2 in range(2):
                    t2sl = slice(th2 * T2, (th2 + 1) * T2)
                    o_acc = oac_p.tile([128, T2, DIM], BF16, tag="oacc")
                    ov = o_acc.rearrange("p t (h d) -> p t h d", h=H)
                    for g in range(H):
                        sgb = (s_all[:, t2sl, g * HD:(g + 1) * HD]
                               .rearrange("p t (o d) -> p t o d", o=1)
                               .to_broadcast([128, T2, H, HD]))
                        agb = (an[:, t2sl, :, g:g + 1]
                               .to_broadcast([128, T2, H, HD]))
                        if g == 0:
                            nc.vector.tensor_mul(ov, sgb, agb)
                        else:
                            tmp = scr.tile([128, T2, H, HD], BF16, tag="ctmp")
                            nc.vector.tensor_mul(tmp, sgb, agb)
                            nc.vector.tensor_add(ov, ov, tmp)
                    for i in range(T2):
                        t = th2 * T2 + i
                        for grp in range(KD // 4):
                            pst = at_ps.tile([128, 4, 128], BF16, tag="atp")
                            for j in range(4):
                                kd = grp * 4 + j
                                nc.tensor.transpose(
                                    pst[:, j, :],
                                    o_acc[:, i, kd * 128:(kd + 1) * 128],
                                    ident_bf)
                            nc.vector.tensor_copy(
                                out=oT[:, grp * 4:(grp + 1) * 4,
                                       t * 128:(t + 1) * 128],
                                in_=pst)

                # --- stage C: proj, merged into res2 on eviction ---
                for g2i in range(4):
                    psp = [[f2_ps.tile([128, CHUNK], F32, tag=f"f2_{m}_{ch}",
                                       name=f"pj_{m}_{ch}_{g2i}")
                            for ch in range(NCH)] for m in range(2)]
                    cols = slice(g2i * 256, (g2i + 1) * 256)
                    for kd in range(KD):
                        wpt = w2str.tile([128, 256], BF16, tag="wpt")
                        nc.sync.dma_start(out=wpt, in_=wp_d.ap()[kd, :, cols])
                        for m in range(2):
                            for ch in range(NCH):
                                nc.tensor.matmul(
                                    psp[m][ch], wpt[:, m * 128:(m + 1) * 128],
                                    oT[:, kd, ch * CHUNK:(ch + 1) * CHUNK],
                                    start=(kd == 0), stop=(kd == KD - 1))
                    for m in range(2):
                        mf = g2i * 2 + m
                        for ch in range(NCH):
                            csl = slice(ch * CHUNK, (ch + 1) * CHUNK)
                            nc.vector.scalar_tensor_tensor(
                                out=res2[:, mf, csl], in0=psp[m][ch],
                                scalar=WSI, in1=res2[:, mf, csl],
                                op0=OP.mult, op1=OP.add)

                # ---------- phase 6: final transpose + residual + store ----
                for t in range(T):
                    yt2 = yts[t]
                    out_tok = outok_p.tile([128, DIM], F32, tag="out_tok")
                    for grp in range(KD // 4):
                        pst = at_ps.tile([128, 4, 128], BF16, tag="atp")
                        for j in range(4):
                            mf = grp * 4 + j
                            nc.tensor.transpose(
                                pst[:, j, :],
                                res2[:, mf, t * 128:(t + 1) * 128], ident_bf)
                        cols = slice(grp * 512, (grp + 1) * 512)
                        nc.vector.tensor_tensor(
                            out=out_tok[:, cols], in0=pst,
                            in1=yt2[:, cols], op=OP.add)
                    nc.sync.dma_start(out=out_r[t], in_=out_tok)

    return nc


MAX_WAITS = 1


def split_big_waits(nc, limit=MAX_WAITS):
    """Walrus rejects instructions carrying too many sem waits; move the
    overflow onto preceding single-wait NoOps on the same engine."""
    n = 0
    for fn in nc.m.functions:
        for blk in fn.blocks:
            new_insts = []
            for inst in blk.instructions:
                si = inst.sync_info
                if si is not None and len(si.on_wait) > limit:
                    waits = list(si.on_wait)
                    while len(waits) > limit:
                        w, waits = waits[0], waits[1:]
                        nop = mybir.InstNoOp(name=f"WSPLIT-{nc.next_id()}")
                        nop.engine = inst.engine
                        nop.sync_info = mybir.SyncInfo(on_wait=[w], on_update=[])
                        new_insts.append(nop)
                        n += 1
                    si.on_wait = waits
                new_insts.append(inst)
            blk.instructions[:] = new_insts
    return n


_NC_CACHE = {}


def get_nc(b_c=B_C):
    """Build + apply the walrus wait-split workaround (HW compile path)."""
    if b_c not in _NC_CACHE:
        nc = build_kernel(b_c)
        split_big_waits(nc)
        _NC_CACHE[b_c] = nc
    return _NC_CACHE[b_c]


def make_in_maps(inputs, b_c=B_C, ncores=NCORES):
    w = prep_weights(
        np.asarray(inputs["Wg"]), np.asarray(inputs["bg"]),
        np.asarray(inputs["Wqkv"]), np.asarray(inputs["Wp"]),
        np.asarray(inputs["bp"]), np.asarray(inputs["g1"]),
        np.asarray(inputs["bn1"]), np.asarray(inputs["g2"]),
        np.asarray(inputs["bn2"]), np.asarray(inputs["W1"]),
        np.asarray(inputs["bm1"]), np.asarray(inputs["W2"]),
        np.asarray(inputs["bm2"]))
    x = f32(np.asarray(inputs["x"]))
    y = f32(np.asarray(inputs["y"]))
    in_maps = []
    for c in range(ncores):
        sl = slice(c * b_c, (c + 1) * b_c)
        in_maps.append({"x": x[sl], "y": y[sl], **w})
    return in_maps


def kernel(**inputs):
    nc = get_nc(B_C)
    in_maps = make_in_maps(inputs)
    res = run_bass_kernel_spmd(nc, in_maps, core_ids=list(range(NCORES)))
    return np.concatenate([res.results[c]["out"] for c in range(NCORES)], axis=0)
